# revision 22
# baseline (speedup 1.0000x reference)
import sys, os, types

sys.path.insert(0, '/opt/trn_rl_repo')

import numpy as np
import ml_dtypes
from concourse import bass, bacc, mybir
import concourse.tile as tile
from concourse.bass_utils import run_bass_kernel_spmd

dt = mybir.dt
F32 = dt.float32
BF16 = dt.bfloat16
AX = mybir.AxisListType
OP = mybir.AluOpType
AF = mybir.ActivationFunctionType

E = 8            # experts == cores
N = 8192         # tokens
C = 768          # model dim
H = 3072         # hidden dim
NCORE = 8
NT = N // 128    # 64 token tiles
NGRP = NT // 8   # 8 groups of 512 slots
CAP = 48         # compacted slots per 128-token tile
PS = 2 * CAP     # slots per pair
GS = 8 * CAP     # slots per group
RS_GROUPS = [[0, 1], [2, 3], [4, 5], [6], [7]]  # groups per RS chunk
NRS = len(RS_GROUPS)


def build_program():
    nc = bacc.Bacc("TRN2", target_bir_lowering=False, debug=False,
                   num_devices=NCORE)

    xTs = nc.dram_tensor("xTs", [C, N // NCORE], F32, kind="ExternalInput")
    xb = nc.dram_tensor("xb", [N, C], BF16, kind="ExternalInput")
    w1 = nc.dram_tensor("w1", [C, H], BF16, kind="ExternalInput")
    w2 = nc.dram_tensor("w2", [H, C], BF16, kind="ExternalInput")
    gw = nc.dram_tensor("gw", [C, E], F32, kind="ExternalInput")
    # constants
    qA = nc.dram_tensor("qA", [128, 128], F32, kind="ExternalInput")   # row iota 0..127
    qB = nc.dram_tensor("qB", [128, 128], F32, kind="ExternalInput")   # row iota -64..63
    ut = nc.dram_tensor("ut", [128, 128], BF16, kind="ExternalInput")  # U[q,p]=1 if q<p
    idn = nc.dram_tensor("idn", [128, 128], F32, kind="ExternalInput")  # identity
    ones = nc.dram_tensor("ones", [128, 1], F32, kind="ExternalInput")
    ecol = nc.dram_tensor("ecol", [128, 1], F32, kind="ExternalInput")  # expert id per core
    io64 = nc.dram_tensor("io64", [128, 64], F32, kind="ExternalInput")  # rows tile(0..7, 8)

    out_sh = nc.dram_tensor("out_sh", [N // 8, C], BF16, kind="ExternalOutput")
    aux_o = nc.dram_tensor("aux_o", [1, 1], F32, kind="ExternalOutput")

    parts = [nc.dram_tensor(f"part{c}", [1024 * len(gs), C], BF16) for c, gs in enumerate(RS_GROUPS)]
    rs_outs = [nc.dram_tensor(f"rs_out{c}", [128 * len(gs), C], BF16)
               for c, gs in enumerate(RS_GROUPS)]
    lgd = nc.dram_tensor("lgd", [128, 64], F32)
    agg = nc.dram_tensor("agg", [128 * NCORE, 64], F32, addr_space="Shared")

    with tile.TileContext(nc) as tc:
        with (
            tc.tile_pool(name="const", bufs=1) as cpool,
            tc.tile_pool(name="wts", bufs=1) as wpool,
            tc.tile_pool(name="rt", bufs=1) as rtpool,        # persistent routing
            tc.tile_pool(name="xg", bufs=4) as xgpool,
            tc.tile_pool(name="xbt", bufs=8) as xbpool,
            tc.tile_pool(name="sm", bufs=4) as smpool,        # small per-group work
            tc.tile_pool(name="pm", bufs=5) as pmpool,        # P matrices
            tc.tile_pool(name="pt", bufs=10) as ptpool,       # scaled P^T (live per group)
            tc.tile_pool(name="xc", bufs=2) as xcpool,        # per-group compact x
            tc.tile_pool(name="ht", bufs=2) as htpool,        # per-group hidden
            tc.tile_pool(name="ysb", bufs=4) as ypool,
            tc.tile_pool(name="osb", bufs=5) as opool,
            tc.tile_pool(name="ps_mm", bufs=3, space="PSUM") as psmm,    # 1-bank units
            tc.tile_pool(name="ps_w", bufs=5, space="PSUM") as psw,      # [*,384] units
        ):
            # ---- load constants / weights ----
            qA_s = cpool.tile([128, 128], F32, tag="qA")
            nc.sync.dma_start(out=qA_s[:], in_=qA[:])
            qB_s = cpool.tile([128, 128], F32, tag="qB")
            nc.sync.dma_start(out=qB_s[:], in_=qB[:])
            ut_s = cpool.tile([128, 128], BF16, tag="ut")
            nc.sync.dma_start(out=ut_s[:], in_=ut[:])
            idn_s = cpool.tile([128, 128], F32, tag="idn")
            nc.sync.dma_start(out=idn_s[:], in_=idn[:])
            ones_s = cpool.tile([128, 1], F32, tag="ones")
            nc.sync.dma_start(out=ones_s[:], in_=ones[:])
            ecol_s = cpool.tile([128, 1], F32, tag="ecol")
            nc.sync.dma_start(out=ecol_s[:], in_=ecol[:])
            io64_s = cpool.tile([128, 64], F32, tag="io64")
            nc.sync.dma_start(out=io64_s[:], in_=io64[:])
            gw_s = cpool.tile([128, 6 * 8], F32, tag="gw")
            nc.sync.dma_start(
                out=gw_s[:].rearrange("p (c e) -> p c e", c=6),
                in_=gw[:].rearrange("(c p) e -> p c e", p=128))

            # ---- persistent routing state ----
            cw_all = rtpool.tile([128, NT], F32, tag="cw")
            mask_all = rtpool.tile([128, NT], F32, tag="mask")
            pos_all = rtpool.tile([128, NT], F32, tag="pos")
            acc_pr = rtpool.tile([128, 64], F32, tag="accp")
            nc.vector.memset(acc_pr[:], 0.0)

            # ---- data-parallel gate: my 1024-token slice, then AllGather ----
            lg_loc = rtpool.tile([128, 64], F32, tag="lgloc")
            for t in range(8):
                xg = xgpool.tile([128, 6 * 128], F32, tag="xg", name=f"xg{t}")
                nc.sync.dma_start(
                    out=xg[:].rearrange("p (c t) -> p c t", c=6),
                    in_=xTs[:].rearrange("(c p) n -> p c n", p=128)
                        [:, :, t * 128:(t + 1) * 128])
                lg_ps = psmm.tile([128, 8], F32, tag="mm", name=f"lgps{t}")
                for cj in range(6):
                    nc.tensor.matmul(
                        out=lg_ps[:],
                        lhsT=xg[:, cj * 128:(cj + 1) * 128],
                        rhs=gw_s[:, cj * 8:(cj + 1) * 8],
                        start=(cj == 0), stop=(cj == 5))
                nc.vector.tensor_copy(out=lg_loc[:, t * 8:(t + 1) * 8], in_=lg_ps[:])
            nc.sync.dma_start(out=lgd[:], in_=lg_loc[:])
            nc.gpsimd.collective_compute(
                "AllGather", OP.bypass,
                ins=[lgd[:]], outs=[agg[:]],
                replica_groups=[list(range(NCORE))])

            w1_s = wpool.tile([128, 6 * H], BF16, tag="w1")
            nc.sync.dma_start(
                out=w1_s[:].rearrange("p (c h) -> p c h", c=6),
                in_=w1[:].rearrange("(c p) h -> p c h", p=128))
            w2_s = wpool.tile([128, 24 * C], BF16, tag="w2")
            nc.sync.dma_start(
                out=w2_s[:].rearrange("p (k c) -> p k c", k=24),
                in_=w2[:].rearrange("(k p) c -> p k c", p=128))


            for g in range(NGRP):
                # ============ routing for this group's 8 tiles ============
                lg_grp = smpool.tile([128, 64], F32, tag="lgg", name=f"lgg{g}")
                nc.sync.dma_start(out=lg_grp[:], in_=agg[g * 128:(g + 1) * 128, :])

                lg3 = lg_grp[:].rearrange("p (t e) -> p t e", t=8)
                m1g = smpool.tile([128, 8], F32, tag="m1g", name=f"m1g{g}")
                nc.vector.reduce_max(out=m1g[:], in_=lg3, axis=AX.X)
                m1b = m1g[:][:, :, None].to_broadcast([128, 8, 8])
                eq1 = smpool.tile([128, 64], F32, tag="eq1", name=f"eq1{g}")
                eq13 = eq1[:].rearrange("p (t e) -> p t e", t=8)
                nc.vector.tensor_tensor(out=eq13, in0=lg3, in1=m1b, op=OP.is_equal)
                t64 = smpool.tile([128, 64], F32, tag="t64", name=f"t64{g}")
                nc.vector.tensor_tensor(out=t64[:], in0=eq1[:], in1=io64_s[:], op=OP.mult)
                a1g = smpool.tile([128, 8], F32, tag="a1g", name=f"a1g{g}")
                nc.vector.reduce_max(out=a1g[:], in_=t64[:].rearrange("p (t e) -> p t e", t=8),
                                     axis=AX.X)
                # mask out argmax, find second
                lm = smpool.tile([128, 64], F32, tag="lm", name=f"lm{g}")
                nc.vector.tensor_scalar(out=lm[:], in0=eq1[:], scalar1=-1e30,
                                        scalar2=None, op0=OP.mult)
                nc.vector.tensor_tensor(out=lm[:], in0=lg_grp[:], in1=lm[:], op=OP.add)
                lm3 = lm[:].rearrange("p (t e) -> p t e", t=8)
                m2g = smpool.tile([128, 8], F32, tag="m2g", name=f"m2g{g}")
                nc.vector.reduce_max(out=m2g[:], in_=lm3, axis=AX.X)
                m2b = m2g[:][:, :, None].to_broadcast([128, 8, 8])
                eq2 = smpool.tile([128, 64], F32, tag="eq2", name=f"eq2{g}")
                nc.vector.tensor_tensor(out=eq2[:].rearrange("p (t e) -> p t e", t=8),
                                        in0=lm3, in1=m2b, op=OP.is_equal)
                nc.vector.tensor_tensor(out=t64[:], in0=eq2[:], in1=io64_s[:], op=OP.mult)
                a2g = smpool.tile([128, 8], F32, tag="a2g", name=f"a2g{g}")
                nc.vector.reduce_max(out=a2g[:], in_=t64[:].rearrange("p (t e) -> p t e", t=8),
                                     axis=AX.X)
                # top-2 softmax weights (bulk ACT)
                d21 = smpool.tile([128, 8], F32, tag="d21", name=f"d21{g}")
                nc.vector.tensor_tensor(out=d21[:], in0=m2g[:], in1=m1g[:], op=OP.subtract)
                w2gg = smpool.tile([128, 8], F32, tag="w2gg", name=f"w2gg{g}")
                nc.scalar.activation(out=w2gg[:], in_=d21[:], func=AF.Sigmoid)
                w1gg = smpool.tile([128, 8], F32, tag="w1gg", name=f"w1gg{g}")
                nc.vector.tensor_scalar(out=w1gg[:], in0=w2gg[:], scalar1=-1.0,
                                        scalar2=1.0, op0=OP.mult, op1=OP.add)
                # softmax probs for aux (bulk)
                zs = smpool.tile([128, 64], F32, tag="zs", name=f"zs{g}")
                nc.vector.tensor_tensor(out=zs[:].rearrange("p (t e) -> p t e", t=8),
                                        in0=lg3, in1=m1b, op=OP.subtract)
                ez = smpool.tile([128, 64], F32, tag="ez", name=f"ez{g}")
                nc.scalar.activation(out=ez[:], in_=zs[:], func=AF.Exp)
                den = smpool.tile([128, 8], F32, tag="den", name=f"den{g}")
                nc.vector.reduce_sum(out=den[:], in_=ez[:].rearrange("p (t e) -> p t e", t=8),
                                     axis=AX.X)
                inv = smpool.tile([128, 8], F32, tag="inv", name=f"inv{g}")
                nc.vector.reciprocal(out=inv[:], in_=den[:])
                invb = inv[:][:, :, None].to_broadcast([128, 8, 8])
                pr = smpool.tile([128, 64], F32, tag="pr", name=f"pr{g}")
                nc.vector.tensor_tensor(out=pr[:].rearrange("p (t e) -> p t e", t=8),
                                        in0=ez[:].rearrange("p (t e) -> p t e", t=8),
                                        in1=invb, op=OP.mult)
                nc.vector.tensor_tensor(out=acc_pr[:], in0=acc_pr[:], in1=pr[:], op=OP.add)
                # my-expert mask + combine weight (bulk [128, 8])
                es1 = smpool.tile([128, 8], F32, tag="es1", name=f"es1{g}")
                nc.vector.tensor_scalar(out=es1[:], in0=a1g[:], scalar1=ecol_s[:, :1],
                                        scalar2=None, op0=OP.is_equal)
                es2 = smpool.tile([128, 8], F32, tag="es2", name=f"es2{g}")
                nc.vector.tensor_scalar(out=es2[:], in0=a2g[:], scalar1=ecol_s[:, :1],
                                        scalar2=None, op0=OP.is_equal)
                nc.vector.tensor_tensor(out=mask_all[:, g * 8:(g + 1) * 8],
                                        in0=es1[:], in1=es2[:], op=OP.add)
                nc.vector.tensor_tensor(out=es1[:], in0=es1[:], in1=w1gg[:], op=OP.mult)
                nc.vector.tensor_tensor(out=es2[:], in0=es2[:], in1=w2gg[:], op=OP.mult)
                nc.vector.tensor_tensor(out=cw_all[:, g * 8:(g + 1) * 8],
                                        in0=es1[:], in1=es2[:], op=OP.add)
                # per-group exclusive prefix (tile-local positions)
                mkbf = smpool.tile([128, 8], BF16, tag="mkbf", name=f"mkbf{g}")
                nc.vector.tensor_copy(out=mkbf[:], in_=mask_all[:, g * 8:(g + 1) * 8])
                cum_ps = psmm.tile([128, 8], F32, tag="mm", name=f"cum{g}")
                nc.tensor.matmul(out=cum_ps[:], lhsT=ut_s[:], rhs=mkbf[:],
                                 start=True, stop=True)
                nc.vector.tensor_copy(out=pos_all[:, g * 8:(g + 1) * 8], in_=cum_ps[:])

                # ============ compaction ============
                xc = xcpool.tile([128, 6 * GS], BF16, tag="xc", name=f"xc{g}")
                pts = []
                for s in range(4):          # pairs in group
                    pair = g * 4 + s
                    cp_ps = [psw.tile([128, 3 * PS], F32, tag="w", name=f"cp{g}_{s}_{i}")
                             for i in range(2)]
                    pmws = []
                    for half in range(2):   # tiles in pair
                        T = pair * 2 + half
                        qio = qA_s if half == 0 else qB_s
                        pmw = pmpool.tile([128, PS], F32, tag="pmw", name=f"pmw{T}")
                        nc.vector.tensor_tensor(
                            out=pmw[:],
                            in0=pos_all[:, T:T + 1].to_broadcast([128, PS]),
                            in1=qio[:, :PS], op=OP.is_equal)
                        nc.vector.tensor_scalar(
                            out=pmw[:], in0=pmw[:], scalar1=mask_all[:, T:T + 1],
                            scalar2=None, op0=OP.mult)
                        pmb = pmpool.tile([128, PS], BF16, tag="pmb", name=f"pmb{T}")
                        nc.vector.tensor_copy(out=pmb[:], in_=pmw[:])
                        pmws.append(pmw)

                        xbt = xbpool.tile([128, 768], BF16, tag="xbt", name=f"xbt{T}")
                        nc.sync.dma_start(out=xbt[:], in_=xb[T * 128:(T + 1) * 128, :])
                        for cj in range(6):
                            # one accumulation group per PSUM bank: start on the
                            # first matmul into the bank, stop on the last.
                            nc.tensor.matmul(
                                out=cp_ps[cj // 3][:, (cj % 3) * PS:(cj % 3 + 1) * PS],
                                lhsT=xbt[:, cj * 128:(cj + 1) * 128],
                                rhs=pmb[:],
                                start=(half == 0 and cj % 3 == 0),
                                stop=(half == 1 and cj % 3 == 2))

                    for half in range(2):
                        T = pair * 2 + half
                        psc = pmpool.tile([128, PS], F32, tag="psc", name=f"psc{T}")
                        nc.vector.tensor_scalar(
                            out=psc[:], in0=pmws[half][:], scalar1=cw_all[:, T:T + 1],
                            scalar2=None, op0=OP.mult)
                        pt_ps = psmm.tile([PS, 128], F32, tag="mm", name=f"ptps{T}")
                        nc.tensor.transpose(out=pt_ps[:], in_=psc[:], identity=idn_s[:])
                        ptb = ptpool.tile([PS, 128], BF16, tag="ptb", name=f"ptb{T}")
                        nc.vector.tensor_copy(out=ptb[:], in_=pt_ps[:])
                        pts.append(ptb)

                    for w in range(2):
                        nc.vector.tensor_copy(
                            out=xc[:].rearrange("p (c q) -> p c q", c=6)
                                [:, w * 3:(w + 1) * 3, s * PS:(s + 1) * PS],
                            in_=cp_ps[w][:].rearrange("p (c q) -> p c q", c=3))

                # ============ GEMM1 + gelu ============
                ht = htpool.tile([128, 24 * GS], BF16, tag="ht", name=f"ht{g}")
                for hk in range(24):
                    g1 = psmm.tile([128, GS], F32, tag="mm", name=f"g1_{g}_{hk}")
                    for cj in range(6):
                        nc.tensor.matmul(
                            out=g1[:],
                            lhsT=w1_s[:, cj * H + hk * 128: cj * H + (hk + 1) * 128],
                            rhs=xc[:, cj * GS:(cj + 1) * GS],
                            start=(cj == 0), stop=(cj == 5))
                    nc.scalar.activation(out=ht[:, hk * GS:(hk + 1) * GS], in_=g1[:],
                                         func=AF.Gelu)

                # ============ GEMM2 + combine ============
                for s in range(4):
                    pair = g * 4 + s
                    y_ps = [psw.tile([PS, 384], F32, tag="w", name=f"y{g}_{s}_{i}")
                            for i in range(2)]
                    for hk in range(24):
                        for w in range(2):
                            nc.tensor.matmul(
                                out=y_ps[w][:, :],
                                lhsT=ht[:, hk * GS + s * PS: hk * GS + (s + 1) * PS],
                                rhs=w2_s[:, hk * C + w * 384: hk * C + (w + 1) * 384],
                                start=(hk == 0), stop=(hk == 23))
                    y_sb = ypool.tile([PS, 768], BF16, tag="ysb", name=f"ysb{pair}")
                    nc.vector.tensor_copy(out=y_sb[:, 0:384], in_=y_ps[0][:])
                    nc.vector.tensor_copy(out=y_sb[:, 384:768], in_=y_ps[1][:])

                    for half in range(2):
                        T = pair * 2 + half
                        ptb = pts[s * 2 + half]
                        oc_ps = [psw.tile([128, 384], F32, tag="w", name=f"oc{T}_{i}")
                                 for i in range(2)]
                        for w in range(2):
                            nc.tensor.matmul(
                                out=oc_ps[w][:],
                                lhsT=ptb[:],
                                rhs=y_sb[:, w * 384:(w + 1) * 384],
                                start=True, stop=True)
                        o_sb = opool.tile([128, 768], BF16, tag="osb", name=f"osb{T}")
                        nc.vector.tensor_copy(out=o_sb[:, 0:384], in_=oc_ps[0][:])
                        nc.vector.tensor_copy(out=o_sb[:, 384:768], in_=oc_ps[1][:])
                        ch = next(i for i, gs in enumerate(RS_GROUPS) if T // 8 in gs)
                        Tc = T - RS_GROUPS[ch][0] * 8
                        nc.sync.dma_start(
                            out=parts[ch][Tc * 128:(Tc + 1) * 128, :],
                            in_=o_sb[:])

                # ============ chunked cross-core combine ============
                for ci, gs in enumerate(RS_GROUPS):
                    if g == gs[-1]:
                        off = RS_GROUPS[ci][0] * 128
                        sz = 128 * len(gs)
                        nc.gpsimd.collective_compute(
                            "ReduceScatter", OP.add,
                            ins=[parts[ci][:]],
                            outs=[rs_outs[ci][:]],
                            replica_groups=[list(range(NCORE))])
                        nc.sync.dma_start(out=out_sh[off:off + sz, :],
                                          in_=rs_outs[ci][:])

            # ---- aux loss finalize ----
            ax_ps = psmm.tile([1, 64], F32, tag="mm")
            nc.tensor.matmul(out=ax_ps[:], lhsT=ones_s[:], rhs=acc_pr[:],
                             start=True, stop=True)
            frv = smpool.tile([1, 64], F32, tag="frv")
            nc.vector.tensor_scalar(out=frv[:], in0=ax_ps[:], scalar1=1.0 / N,
                                    scalar2=None, op0=OP.mult)
            fre = smpool.tile([1, 8], F32, tag="fre")
            nc.vector.reduce_sum(out=fre[:], in_=frv[:].rearrange("p (t e) -> p e t", t=8),
                                 axis=AX.X)
            nc.vector.tensor_tensor(out=fre[:], in0=fre[:], in1=fre[:], op=OP.mult)
            axv = smpool.tile([1, 1], F32, tag="axv")
            nc.vector.reduce_sum(out=axv[:], in_=fre[:], axis=AX.X)
            nc.vector.tensor_scalar(out=axv[:], in0=axv[:], scalar1=0.01 * E,
                                    scalar2=None, op0=OP.mult)
            nc.sync.dma_start(out=aux_o[:], in_=axv[:])



    nc.compile()
    return nc


def make_consts():
    qA = np.tile(np.arange(128, dtype=np.float32), (128, 1))
    qB = qA - float(CAP)
    ut = np.triu(np.ones((128, 128), dtype=np.float32), k=1)
    idn = np.eye(128, dtype=np.float32)
    ones = np.ones((128, 1), dtype=np.float32)
    io64 = np.tile(np.tile(np.arange(8, dtype=np.float32), 8), (128, 1))
    return qA, qB, ut, idn, ones, io64


def kernel(x, gate_w, w1, w2):
    x = np.asarray(x, dtype=np.float32)
    gate_w = np.asarray(gate_w, dtype=np.float32)
    w1 = np.asarray(w1, dtype=np.float32)
    w2 = np.asarray(w2, dtype=np.float32)
    B, T_, C_ = x.shape
    xt = np.ascontiguousarray(x.reshape(-1, C_))

    # capacity sanity check (cheap host-side gate replication)
    lgh = xt @ gate_w
    top2 = np.argpartition(-lgh, 2, axis=1)[:, :2]
    cnt = np.zeros((NT, E), np.int64)
    for k in range(2):
        np.add.at(cnt, (np.arange(N) // 128, top2[:, k]), 1)
    assert cnt.max() <= CAP, f"tile capacity overflow: {cnt.max()} > {CAP}"

    xT = np.ascontiguousarray(xt.T)
    xTslices = [np.ascontiguousarray(xT[:, 1024 * e:1024 * (e + 1)]) for e in range(NCORE)]
    xbf = xt.astype(ml_dtypes.bfloat16)
    w1b = w1.astype(ml_dtypes.bfloat16)
    w2b = w2.astype(ml_dtypes.bfloat16)

    qA, qB, ut, idn, ones, io64 = make_consts()
    utb = ut.astype(ml_dtypes.bfloat16)

    nc = build_program()

    in_maps = []
    for e in range(NCORE):
        in_maps.append({
            "xTs": xTslices[e], "xb": xbf,
            "w1": np.ascontiguousarray(w1b[e]),
            "w2": np.ascontiguousarray(w2b[e]),
            "gw": gate_w,
            "qA": qA, "qB": qB, "ut": utb, "idn": idn, "ones": ones,
            "ecol": np.full((128, 1), float(e), dtype=np.float32),
            "io64": io64,
        })

    res = run_bass_kernel_spmd(nc, in_maps, core_ids=list(range(NCORE)),
                               **_trace_kwargs())
    _stash_results(res)

    # reassemble: chunked RS gives core i, chunk c -> global rows 2048c+256i..+256
    out = np.empty((N, C), np.float32)
    for i in range(NCORE):
        sh = res.results[i]["out_sh"]
        for gs in RS_GROUPS:
            base = gs[0] * 1024              # global row of chunk start
            csz = 1024 * len(gs) // NCORE    # rows per core in this chunk
            shoff = gs[0] * 128              # rows into rs_out/out_sh
            out[base + csz * i: base + csz * (i + 1)] = \
                np.asarray(sh[shoff: shoff + csz], dtype=np.float32)
    out = out.reshape(B, T_, C_)
    aux = np.float32(res.results[0]["aux_o"][0, 0])
    return out, aux


_TRACE = {"enabled": False}
_LAST = {}


def _trace_kwargs():
    if not _TRACE["enabled"]:
        return {}
    import concourse.bass_utils as bu
    bu.upload_artifacts = lambda d: d
    m = types.ModuleType("antenv.axon_hooks")
    m._hook = None
    def set_hook(h): m._hook = h
    def get_hook(): return m._hook
    m.set_axon_ntff_profile_hook = set_hook
    m.get_axon_ntff_profile_hook = get_hook
    sys.modules['antenv.axon_hooks'] = m
    import antenv
    antenv.axon_hooks = m
    from trn_agent_boot.trn_boot import _ntff_profile_via_ctypes
    set_hook(_ntff_profile_via_ctypes('/opt/axon/libaxon_pjrt.so'))
    return {"trace": True}


def _stash_results(res):
    _LAST["exec_time_ns"] = res.exec_time_ns
    _LAST["mean_exec_time_ns"] = res.mean_exec_time_ns
    if res.instructions_and_trace:
        _LAST["trace_path"] = res.instructions_and_trace[1]


# revision 23
# speedup vs baseline: 1.0381x; 1.0381x over previous
import sys, os, types

sys.path.insert(0, '/opt/trn_rl_repo')

import numpy as np
import ml_dtypes
from concourse import bass, bacc, mybir
import concourse.tile as tile
from concourse.bass_utils import run_bass_kernel_spmd

dt = mybir.dt
F32 = dt.float32
BF16 = dt.bfloat16
AX = mybir.AxisListType
OP = mybir.AluOpType
AF = mybir.ActivationFunctionType

E = 8            # experts == cores
N = 8192         # tokens
C = 768          # model dim
H = 3072         # hidden dim
NCORE = 8
NT = N // 128    # 64 token tiles
NGRP = NT // 8   # 8 groups of 512 slots
CAP = 48         # compacted slots per 128-token tile
PS = 2 * CAP     # slots per pair
GS = 8 * CAP     # slots per group
RS_GROUPS = [[0, 1], [2, 3], [4, 5], [6], [7]]  # groups per RS chunk
NRS = len(RS_GROUPS)


def build_program():
    nc = bacc.Bacc("TRN2", target_bir_lowering=False, debug=False,
                   num_devices=NCORE)

    xTs = nc.dram_tensor("xTs", [C, N // NCORE], F32, kind="ExternalInput")
    xb = nc.dram_tensor("xb", [N, C], BF16, kind="ExternalInput")
    w1 = nc.dram_tensor("w1", [C, H], BF16, kind="ExternalInput")
    w2 = nc.dram_tensor("w2", [H, C], BF16, kind="ExternalInput")
    gw = nc.dram_tensor("gw", [C, E], F32, kind="ExternalInput")
    # constants
    qA = nc.dram_tensor("qA", [128, 128], F32, kind="ExternalInput")   # row iota 0..127
    qB = nc.dram_tensor("qB", [128, 128], F32, kind="ExternalInput")   # row iota -64..63
    ut = nc.dram_tensor("ut", [128, 128], BF16, kind="ExternalInput")  # U[q,p]=1 if q<p
    idn = nc.dram_tensor("idn", [128, 128], F32, kind="ExternalInput")  # identity
    ones = nc.dram_tensor("ones", [128, 1], F32, kind="ExternalInput")
    ecol = nc.dram_tensor("ecol", [128, 1], F32, kind="ExternalInput")  # expert id per core
    io64 = nc.dram_tensor("io64", [128, 64], F32, kind="ExternalInput")  # rows tile(0..7, 8)

    out_sh = nc.dram_tensor("out_sh", [N // 8, C], BF16, kind="ExternalOutput")
    aux_o = nc.dram_tensor("aux_o", [1, 1], F32, kind="ExternalOutput")

    parts = [nc.dram_tensor(f"part{c}", [1024 * len(gs), C], BF16) for c, gs in enumerate(RS_GROUPS)]
    rs_outs = [nc.dram_tensor(f"rs_out{c}", [128 * len(gs), C], BF16)
               for c, gs in enumerate(RS_GROUPS)]
    lgd = nc.dram_tensor("lgd", [128, 64], F32)
    agg = nc.dram_tensor("agg", [128 * NCORE, 64], F32, addr_space="Shared")

    with tile.TileContext(nc) as tc:
        with (
            tc.tile_pool(name="const", bufs=1) as cpool,
            tc.tile_pool(name="wts", bufs=1) as wpool,
            tc.tile_pool(name="rt", bufs=1) as rtpool,        # persistent routing
            tc.tile_pool(name="xg", bufs=5) as xgpool,
            tc.tile_pool(name="xbt", bufs=10) as xbpool,
            tc.tile_pool(name="sm", bufs=4) as smpool,        # small per-group work
            tc.tile_pool(name="pm", bufs=4) as pmpool,        # P matrices
            tc.tile_pool(name="pt", bufs=10) as ptpool,       # scaled P^T (live per group)
            tc.tile_pool(name="xc", bufs=2) as xcpool,        # per-group compact x
            tc.tile_pool(name="ht", bufs=2) as htpool,        # per-group hidden
            tc.tile_pool(name="ysb", bufs=3) as ypool,
            tc.tile_pool(name="osb", bufs=3) as opool,
            tc.tile_pool(name="ps_mm", bufs=3, space="PSUM") as psmm,    # 1-bank units
            tc.tile_pool(name="ps_w", bufs=5, space="PSUM") as psw,      # [*,384] units
        ):
            # ---- load constants / weights ----
            qA_s = cpool.tile([128, 128], F32, tag="qA")
            nc.sync.dma_start(out=qA_s[:], in_=qA[:])
            qB_s = cpool.tile([128, 128], F32, tag="qB")
            nc.sync.dma_start(out=qB_s[:], in_=qB[:])
            ut_s = cpool.tile([128, 128], BF16, tag="ut")
            nc.sync.dma_start(out=ut_s[:], in_=ut[:])
            idn_s = cpool.tile([128, 128], F32, tag="idn")
            nc.sync.dma_start(out=idn_s[:], in_=idn[:])
            ones_s = cpool.tile([128, 1], F32, tag="ones")
            nc.sync.dma_start(out=ones_s[:], in_=ones[:])
            ecol_s = cpool.tile([128, 1], F32, tag="ecol")
            nc.sync.dma_start(out=ecol_s[:], in_=ecol[:])
            io64_s = cpool.tile([128, 64], F32, tag="io64")
            nc.sync.dma_start(out=io64_s[:], in_=io64[:])
            gw_s = cpool.tile([128, 6 * 8], F32, tag="gw")
            nc.sync.dma_start(
                out=gw_s[:].rearrange("p (c e) -> p c e", c=6),
                in_=gw[:].rearrange("(c p) e -> p c e", p=128))

            # ---- persistent routing state ----
            cw_all = rtpool.tile([128, NT], F32, tag="cw")
            mask_all = rtpool.tile([128, NT], F32, tag="mask")
            pos_all = rtpool.tile([128, NT], F32, tag="pos")
            acc_pr = rtpool.tile([128, 64], F32, tag="accp")
            nc.vector.memset(acc_pr[:], 0.0)

            # ---- data-parallel gate: my 1024-token slice, then AllGather ----
            lg_loc = rtpool.tile([128, 64], F32, tag="lgloc")
            for t in range(8):
                xg = xgpool.tile([128, 6 * 128], F32, tag="xg", name=f"xg{t}")
                nc.sync.dma_start(
                    out=xg[:].rearrange("p (c t) -> p c t", c=6),
                    in_=xTs[:].rearrange("(c p) n -> p c n", p=128)
                        [:, :, t * 128:(t + 1) * 128])
                lg_ps = psmm.tile([128, 8], F32, tag="mm", name=f"lgps{t}")
                for cj in range(6):
                    nc.tensor.matmul(
                        out=lg_ps[:],
                        lhsT=xg[:, cj * 128:(cj + 1) * 128],
                        rhs=gw_s[:, cj * 8:(cj + 1) * 8],
                        start=(cj == 0), stop=(cj == 5))
                nc.vector.tensor_copy(out=lg_loc[:, t * 8:(t + 1) * 8], in_=lg_ps[:])
            nc.sync.dma_start(out=lgd[:], in_=lg_loc[:])
            nc.gpsimd.collective_compute(
                "AllGather", OP.bypass,
                ins=[lgd[:]], outs=[agg[:]],
                replica_groups=[list(range(NCORE))])

            w1_s = wpool.tile([128, 6 * H], BF16, tag="w1")
            nc.sync.dma_start(
                out=w1_s[:].rearrange("p (c h) -> p c h", c=6),
                in_=w1[:].rearrange("(c p) h -> p c h", p=128))
            w2_s = wpool.tile([128, 24 * C], BF16, tag="w2")
            nc.sync.dma_start(
                out=w2_s[:].rearrange("p (k c) -> p k c", k=24),
                in_=w2[:].rearrange("(k p) c -> p k c", p=128))


            for g in range(NGRP):
                # ============ routing for this group's 8 tiles ============
                lg_grp = smpool.tile([128, 64], F32, tag="lgg", name=f"lgg{g}")
                nc.sync.dma_start(out=lg_grp[:], in_=agg[g * 128:(g + 1) * 128, :])

                lg3 = lg_grp[:].rearrange("p (t e) -> p t e", t=8)
                m1g = smpool.tile([128, 8], F32, tag="m1g", name=f"m1g{g}")
                nc.vector.reduce_max(out=m1g[:], in_=lg3, axis=AX.X)
                m1b = m1g[:][:, :, None].to_broadcast([128, 8, 8])
                eq1 = smpool.tile([128, 64], F32, tag="eq1", name=f"eq1{g}")
                eq13 = eq1[:].rearrange("p (t e) -> p t e", t=8)
                nc.vector.tensor_tensor(out=eq13, in0=lg3, in1=m1b, op=OP.is_equal)
                t64 = smpool.tile([128, 64], F32, tag="t64", name=f"t64{g}")
                nc.vector.tensor_tensor(out=t64[:], in0=eq1[:], in1=io64_s[:], op=OP.mult)
                a1g = smpool.tile([128, 8], F32, tag="a1g", name=f"a1g{g}")
                nc.vector.reduce_max(out=a1g[:], in_=t64[:].rearrange("p (t e) -> p t e", t=8),
                                     axis=AX.X)
                # mask out argmax, find second
                lm = smpool.tile([128, 64], F32, tag="lm", name=f"lm{g}")
                nc.vector.tensor_scalar(out=lm[:], in0=eq1[:], scalar1=-1e30,
                                        scalar2=None, op0=OP.mult)
                nc.vector.tensor_tensor(out=lm[:], in0=lg_grp[:], in1=lm[:], op=OP.add)
                lm3 = lm[:].rearrange("p (t e) -> p t e", t=8)
                m2g = smpool.tile([128, 8], F32, tag="m2g", name=f"m2g{g}")
                nc.vector.reduce_max(out=m2g[:], in_=lm3, axis=AX.X)
                m2b = m2g[:][:, :, None].to_broadcast([128, 8, 8])
                eq2 = smpool.tile([128, 64], F32, tag="eq2", name=f"eq2{g}")
                nc.vector.tensor_tensor(out=eq2[:].rearrange("p (t e) -> p t e", t=8),
                                        in0=lm3, in1=m2b, op=OP.is_equal)
                nc.vector.tensor_tensor(out=t64[:], in0=eq2[:], in1=io64_s[:], op=OP.mult)
                a2g = smpool.tile([128, 8], F32, tag="a2g", name=f"a2g{g}")
                nc.vector.reduce_max(out=a2g[:], in_=t64[:].rearrange("p (t e) -> p t e", t=8),
                                     axis=AX.X)
                # top-2 softmax weights (bulk ACT)
                d21 = smpool.tile([128, 8], F32, tag="d21", name=f"d21{g}")
                nc.vector.tensor_tensor(out=d21[:], in0=m2g[:], in1=m1g[:], op=OP.subtract)
                w2gg = smpool.tile([128, 8], F32, tag="w2gg", name=f"w2gg{g}")
                nc.scalar.activation(out=w2gg[:], in_=d21[:], func=AF.Sigmoid)
                w1gg = smpool.tile([128, 8], F32, tag="w1gg", name=f"w1gg{g}")
                nc.vector.tensor_scalar(out=w1gg[:], in0=w2gg[:], scalar1=-1.0,
                                        scalar2=1.0, op0=OP.mult, op1=OP.add)
                # softmax probs for aux (bulk)
                zs = smpool.tile([128, 64], F32, tag="zs", name=f"zs{g}")
                nc.vector.tensor_tensor(out=zs[:].rearrange("p (t e) -> p t e", t=8),
                                        in0=lg3, in1=m1b, op=OP.subtract)
                ez = smpool.tile([128, 64], F32, tag="ez", name=f"ez{g}")
                nc.scalar.activation(out=ez[:], in_=zs[:], func=AF.Exp)
                den = smpool.tile([128, 8], F32, tag="den", name=f"den{g}")
                nc.vector.reduce_sum(out=den[:], in_=ez[:].rearrange("p (t e) -> p t e", t=8),
                                     axis=AX.X)
                inv = smpool.tile([128, 8], F32, tag="inv", name=f"inv{g}")
                nc.vector.reciprocal(out=inv[:], in_=den[:])
                invb = inv[:][:, :, None].to_broadcast([128, 8, 8])
                pr = smpool.tile([128, 64], F32, tag="pr", name=f"pr{g}")
                nc.vector.tensor_tensor(out=pr[:].rearrange("p (t e) -> p t e", t=8),
                                        in0=ez[:].rearrange("p (t e) -> p t e", t=8),
                                        in1=invb, op=OP.mult)
                nc.vector.tensor_tensor(out=acc_pr[:], in0=acc_pr[:], in1=pr[:], op=OP.add)
                # my-expert mask + combine weight (bulk [128, 8])
                es1 = smpool.tile([128, 8], F32, tag="es1", name=f"es1{g}")
                nc.vector.tensor_scalar(out=es1[:], in0=a1g[:], scalar1=ecol_s[:, :1],
                                        scalar2=None, op0=OP.is_equal)
                es2 = smpool.tile([128, 8], F32, tag="es2", name=f"es2{g}")
                nc.vector.tensor_scalar(out=es2[:], in0=a2g[:], scalar1=ecol_s[:, :1],
                                        scalar2=None, op0=OP.is_equal)
                nc.vector.tensor_tensor(out=mask_all[:, g * 8:(g + 1) * 8],
                                        in0=es1[:], in1=es2[:], op=OP.add)
                nc.vector.tensor_tensor(out=es1[:], in0=es1[:], in1=w1gg[:], op=OP.mult)
                nc.vector.tensor_tensor(out=es2[:], in0=es2[:], in1=w2gg[:], op=OP.mult)
                nc.vector.tensor_tensor(out=cw_all[:, g * 8:(g + 1) * 8],
                                        in0=es1[:], in1=es2[:], op=OP.add)
                # per-group exclusive prefix (tile-local positions)
                mkbf = smpool.tile([128, 8], BF16, tag="mkbf", name=f"mkbf{g}")
                nc.vector.tensor_copy(out=mkbf[:], in_=mask_all[:, g * 8:(g + 1) * 8])
                cum_ps = psmm.tile([128, 8], F32, tag="mm", name=f"cum{g}")
                nc.tensor.matmul(out=cum_ps[:], lhsT=ut_s[:], rhs=mkbf[:],
                                 start=True, stop=True)
                nc.vector.tensor_copy(out=pos_all[:, g * 8:(g + 1) * 8], in_=cum_ps[:])

                # ============ compaction ============
                xc = xcpool.tile([128, 6 * GS], BF16, tag="xc", name=f"xc{g}")
                pts = []
                for s in range(4):          # pairs in group
                    pair = g * 4 + s
                    cp_ps = [psw.tile([128, 3 * PS], F32, tag="w", name=f"cp{g}_{s}_{i}")
                             for i in range(2)]
                    pmws = []
                    for half in range(2):   # tiles in pair
                        T = pair * 2 + half
                        qio = qA_s if half == 0 else qB_s
                        pmw = pmpool.tile([128, PS], F32, tag="pmw", name=f"pmw{T}")
                        nc.vector.tensor_tensor(
                            out=pmw[:],
                            in0=pos_all[:, T:T + 1].to_broadcast([128, PS]),
                            in1=qio[:, :PS], op=OP.is_equal)
                        nc.vector.tensor_scalar(
                            out=pmw[:], in0=pmw[:], scalar1=mask_all[:, T:T + 1],
                            scalar2=None, op0=OP.mult)
                        pmb = pmpool.tile([128, PS], BF16, tag="pmb", name=f"pmb{T}")
                        nc.vector.tensor_copy(out=pmb[:], in_=pmw[:])
                        pmws.append(pmw)

                        xbt = xbpool.tile([128, 768], BF16, tag="xbt", name=f"xbt{T}")
                        nc.sync.dma_start(out=xbt[:], in_=xb[T * 128:(T + 1) * 128, :])
                        for cj in range(6):
                            # one accumulation group per PSUM bank: start on the
                            # first matmul into the bank, stop on the last.
                            nc.tensor.matmul(
                                out=cp_ps[cj // 3][:, (cj % 3) * PS:(cj % 3 + 1) * PS],
                                lhsT=xbt[:, cj * 128:(cj + 1) * 128],
                                rhs=pmb[:],
                                start=(half == 0 and cj % 3 == 0),
                                stop=(half == 1 and cj % 3 == 2))

                    for half in range(2):
                        T = pair * 2 + half
                        psc = pmpool.tile([128, PS], F32, tag="psc", name=f"psc{T}")
                        nc.vector.tensor_scalar(
                            out=psc[:], in0=pmws[half][:], scalar1=cw_all[:, T:T + 1],
                            scalar2=None, op0=OP.mult)
                        pt_ps = psmm.tile([PS, 128], F32, tag="mm", name=f"ptps{T}")
                        nc.tensor.transpose(out=pt_ps[:], in_=psc[:], identity=idn_s[:])
                        ptb = ptpool.tile([PS, 128], BF16, tag="ptb", name=f"ptb{T}")
                        nc.vector.tensor_copy(out=ptb[:], in_=pt_ps[:])
                        pts.append(ptb)

                    for w in range(2):
                        nc.vector.tensor_copy(
                            out=xc[:].rearrange("p (c q) -> p c q", c=6)
                                [:, w * 3:(w + 1) * 3, s * PS:(s + 1) * PS],
                            in_=cp_ps[w][:].rearrange("p (c q) -> p c q", c=3))

                # ============ GEMM1 + gelu ============
                ht = htpool.tile([128, 24 * GS], BF16, tag="ht", name=f"ht{g}")
                for hk in range(24):
                    g1 = psmm.tile([128, GS], F32, tag="mm", name=f"g1_{g}_{hk}")
                    for cj in range(6):
                        nc.tensor.matmul(
                            out=g1[:],
                            lhsT=w1_s[:, cj * H + hk * 128: cj * H + (hk + 1) * 128],
                            rhs=xc[:, cj * GS:(cj + 1) * GS],
                            start=(cj == 0), stop=(cj == 5))
                    nc.scalar.activation(out=ht[:, hk * GS:(hk + 1) * GS], in_=g1[:],
                                         func=AF.Gelu)

                # ============ GEMM2 + combine ============
                for s in range(4):
                    pair = g * 4 + s
                    y_ps = [psw.tile([PS, 384], F32, tag="w", name=f"y{g}_{s}_{i}")
                            for i in range(2)]
                    for hk in range(24):
                        for w in range(2):
                            nc.tensor.matmul(
                                out=y_ps[w][:, :],
                                lhsT=ht[:, hk * GS + s * PS: hk * GS + (s + 1) * PS],
                                rhs=w2_s[:, hk * C + w * 384: hk * C + (w + 1) * 384],
                                start=(hk == 0), stop=(hk == 23))
                    y_sb = ypool.tile([PS, 768], BF16, tag="ysb", name=f"ysb{pair}")
                    nc.vector.tensor_copy(out=y_sb[:, 0:384], in_=y_ps[0][:])
                    nc.vector.tensor_copy(out=y_sb[:, 384:768], in_=y_ps[1][:])

                    for half in range(2):
                        T = pair * 2 + half
                        ptb = pts[s * 2 + half]
                        oc_ps = [psw.tile([128, 384], F32, tag="w", name=f"oc{T}_{i}")
                                 for i in range(2)]
                        for w in range(2):
                            nc.tensor.matmul(
                                out=oc_ps[w][:],
                                lhsT=ptb[:],
                                rhs=y_sb[:, w * 384:(w + 1) * 384],
                                start=True, stop=True)
                        o_sb = opool.tile([128, 768], BF16, tag="osb", name=f"osb{T}")
                        nc.vector.tensor_copy(out=o_sb[:, 0:384], in_=oc_ps[0][:])
                        nc.vector.tensor_copy(out=o_sb[:, 384:768], in_=oc_ps[1][:])
                        ch = next(i for i, gs in enumerate(RS_GROUPS) if T // 8 in gs)
                        Tc = T - RS_GROUPS[ch][0] * 8
                        nc.sync.dma_start(
                            out=parts[ch][Tc * 128:(Tc + 1) * 128, :],
                            in_=o_sb[:])

                # ============ chunked cross-core combine ============
                for ci, gs in enumerate(RS_GROUPS):
                    if g == gs[-1]:
                        off = RS_GROUPS[ci][0] * 128
                        sz = 128 * len(gs)
                        nc.gpsimd.collective_compute(
                            "ReduceScatter", OP.add,
                            ins=[parts[ci][:]],
                            outs=[rs_outs[ci][:]],
                            replica_groups=[list(range(NCORE))])
                        nc.sync.dma_start(out=out_sh[off:off + sz, :],
                                          in_=rs_outs[ci][:])

            # ---- aux loss finalize ----
            ax_ps = psmm.tile([1, 64], F32, tag="mm")
            nc.tensor.matmul(out=ax_ps[:], lhsT=ones_s[:], rhs=acc_pr[:],
                             start=True, stop=True)
            frv = smpool.tile([1, 64], F32, tag="frv")
            nc.vector.tensor_scalar(out=frv[:], in0=ax_ps[:], scalar1=1.0 / N,
                                    scalar2=None, op0=OP.mult)
            fre = smpool.tile([1, 8], F32, tag="fre")
            nc.vector.reduce_sum(out=fre[:], in_=frv[:].rearrange("p (t e) -> p e t", t=8),
                                 axis=AX.X)
            nc.vector.tensor_tensor(out=fre[:], in0=fre[:], in1=fre[:], op=OP.mult)
            axv = smpool.tile([1, 1], F32, tag="axv")
            nc.vector.reduce_sum(out=axv[:], in_=fre[:], axis=AX.X)
            nc.vector.tensor_scalar(out=axv[:], in0=axv[:], scalar1=0.01 * E,
                                    scalar2=None, op0=OP.mult)
            nc.sync.dma_start(out=aux_o[:], in_=axv[:])



    nc.compile()
    return nc


def make_consts():
    qA = np.tile(np.arange(128, dtype=np.float32), (128, 1))
    qB = qA - float(CAP)
    ut = np.triu(np.ones((128, 128), dtype=np.float32), k=1)
    idn = np.eye(128, dtype=np.float32)
    ones = np.ones((128, 1), dtype=np.float32)
    io64 = np.tile(np.tile(np.arange(8, dtype=np.float32), 8), (128, 1))
    return qA, qB, ut, idn, ones, io64


def kernel(x, gate_w, w1, w2):
    x = np.asarray(x, dtype=np.float32)
    gate_w = np.asarray(gate_w, dtype=np.float32)
    w1 = np.asarray(w1, dtype=np.float32)
    w2 = np.asarray(w2, dtype=np.float32)
    B, T_, C_ = x.shape
    xt = np.ascontiguousarray(x.reshape(-1, C_))

    # capacity sanity check (cheap host-side gate replication)
    lgh = xt @ gate_w
    top2 = np.argpartition(-lgh, 2, axis=1)[:, :2]
    cnt = np.zeros((NT, E), np.int64)
    for k in range(2):
        np.add.at(cnt, (np.arange(N) // 128, top2[:, k]), 1)
    assert cnt.max() <= CAP, f"tile capacity overflow: {cnt.max()} > {CAP}"

    xT = np.ascontiguousarray(xt.T)
    xTslices = [np.ascontiguousarray(xT[:, 1024 * e:1024 * (e + 1)]) for e in range(NCORE)]
    xbf = xt.astype(ml_dtypes.bfloat16)
    w1b = w1.astype(ml_dtypes.bfloat16)
    w2b = w2.astype(ml_dtypes.bfloat16)

    qA, qB, ut, idn, ones, io64 = make_consts()
    utb = ut.astype(ml_dtypes.bfloat16)

    nc = build_program()

    in_maps = []
    for e in range(NCORE):
        in_maps.append({
            "xTs": xTslices[e], "xb": xbf,
            "w1": np.ascontiguousarray(w1b[e]),
            "w2": np.ascontiguousarray(w2b[e]),
            "gw": gate_w,
            "qA": qA, "qB": qB, "ut": utb, "idn": idn, "ones": ones,
            "ecol": np.full((128, 1), float(e), dtype=np.float32),
            "io64": io64,
        })

    res = run_bass_kernel_spmd(nc, in_maps, core_ids=list(range(NCORE)),
                               **_trace_kwargs())
    _stash_results(res)

    # reassemble: chunked RS gives core i, chunk c -> global rows 2048c+256i..+256
    out = np.empty((N, C), np.float32)
    for i in range(NCORE):
        sh = res.results[i]["out_sh"]
        for gs in RS_GROUPS:
            base = gs[0] * 1024              # global row of chunk start
            csz = 1024 * len(gs) // NCORE    # rows per core in this chunk
            shoff = gs[0] * 128              # rows into rs_out/out_sh
            out[base + csz * i: base + csz * (i + 1)] = \
                np.asarray(sh[shoff: shoff + csz], dtype=np.float32)
    out = out.reshape(B, T_, C_)
    aux = np.float32(res.results[0]["aux_o"][0, 0])
    return out, aux


_TRACE = {"enabled": False}
_LAST = {}


def _trace_kwargs():
    if not _TRACE["enabled"]:
        return {}
    import concourse.bass_utils as bu
    bu.upload_artifacts = lambda d: d
    m = types.ModuleType("antenv.axon_hooks")
    m._hook = None
    def set_hook(h): m._hook = h
    def get_hook(): return m._hook
    m.set_axon_ntff_profile_hook = set_hook
    m.get_axon_ntff_profile_hook = get_hook
    sys.modules['antenv.axon_hooks'] = m
    import antenv
    antenv.axon_hooks = m
    from trn_agent_boot.trn_boot import _ntff_profile_via_ctypes
    set_hook(_ntff_profile_via_ctypes('/opt/axon/libaxon_pjrt.so'))
    return {"trace": True}


def _stash_results(res):
    _LAST["exec_time_ns"] = res.exec_time_ns
    _LAST["mean_exec_time_ns"] = res.mean_exec_time_ns
    if res.instructions_and_trace:
        _LAST["trace_path"] = res.instructions_and_trace[1]


# revision 25
# speedup vs baseline: 1.0453x; 1.0070x over previous
import sys, os, types

sys.path.insert(0, '/opt/trn_rl_repo')

import numpy as np
import ml_dtypes
from concourse import bass, bacc, mybir
import concourse.tile as tile
from concourse.bass_utils import run_bass_kernel_spmd

dt = mybir.dt
F32 = dt.float32
BF16 = dt.bfloat16
AX = mybir.AxisListType
OP = mybir.AluOpType
AF = mybir.ActivationFunctionType

E = 8            # experts == cores
N = 8192         # tokens
C = 768          # model dim
H = 3072         # hidden dim
NCORE = 8
NT = N // 128    # 64 token tiles
NGRP = NT // 8   # 8 groups of 512 slots
CAP = 48         # compacted slots per 128-token tile
PS = 2 * CAP     # slots per pair
GS = 8 * CAP     # slots per group
RS_GROUPS = [[0], [1, 2], [3, 4], [5, 6], [7]]  # groups per RS chunk
NRS = len(RS_GROUPS)


def build_program():
    nc = bacc.Bacc("TRN2", target_bir_lowering=False, debug=False,
                   num_devices=NCORE)

    xTs = nc.dram_tensor("xTs", [C, N // NCORE], F32, kind="ExternalInput")
    xb = nc.dram_tensor("xb", [N, C], BF16, kind="ExternalInput")
    w1 = nc.dram_tensor("w1", [C, H], BF16, kind="ExternalInput")
    w2 = nc.dram_tensor("w2", [H, C], BF16, kind="ExternalInput")
    gw = nc.dram_tensor("gw", [C, E], F32, kind="ExternalInput")
    # constants
    qA = nc.dram_tensor("qA", [128, 128], F32, kind="ExternalInput")   # row iota 0..127
    qB = nc.dram_tensor("qB", [128, 128], F32, kind="ExternalInput")   # row iota -64..63
    ut = nc.dram_tensor("ut", [128, 128], BF16, kind="ExternalInput")  # U[q,p]=1 if q<p
    idn = nc.dram_tensor("idn", [128, 128], F32, kind="ExternalInput")  # identity
    ones = nc.dram_tensor("ones", [128, 1], F32, kind="ExternalInput")
    ecol = nc.dram_tensor("ecol", [128, 1], F32, kind="ExternalInput")  # expert id per core
    io64 = nc.dram_tensor("io64", [128, 64], F32, kind="ExternalInput")  # rows tile(0..7, 8)

    out_sh = nc.dram_tensor("out_sh", [N // 8, C], BF16, kind="ExternalOutput")
    aux_o = nc.dram_tensor("aux_o", [1, 1], F32, kind="ExternalOutput")

    parts = [nc.dram_tensor(f"part{c}", [1024 * len(gs), C], BF16) for c, gs in enumerate(RS_GROUPS)]
    rs_outs = [nc.dram_tensor(f"rs_out{c}", [128 * len(gs), C], BF16)
               for c, gs in enumerate(RS_GROUPS)]
    lgd = nc.dram_tensor("lgd", [128, 64], F32)
    agg = nc.dram_tensor("agg", [128 * NCORE, 64], F32, addr_space="Shared")

    with tile.TileContext(nc) as tc:
        with (
            tc.tile_pool(name="const", bufs=1) as cpool,
            tc.tile_pool(name="wts", bufs=1) as wpool,
            tc.tile_pool(name="rt", bufs=1) as rtpool,        # persistent routing
            tc.tile_pool(name="xg", bufs=4) as xgpool,
            tc.tile_pool(name="xbt", bufs=8) as xbpool,
            tc.tile_pool(name="sm", bufs=4) as smpool,        # small per-group work
            tc.tile_pool(name="pm", bufs=4) as pmpool,        # P matrices
            tc.tile_pool(name="pt", bufs=10) as ptpool,       # scaled P^T (live per group)
            tc.tile_pool(name="xc", bufs=2) as xcpool,        # per-group compact x
            tc.tile_pool(name="ht", bufs=2) as htpool,        # per-group hidden
            tc.tile_pool(name="ysb", bufs=3) as ypool,
            tc.tile_pool(name="osb", bufs=3) as opool,
            tc.tile_pool(name="ps_mm", bufs=3, space="PSUM") as psmm,    # 1-bank units
            tc.tile_pool(name="ps_w", bufs=5, space="PSUM") as psw,      # [*,384] units
        ):
            # ---- load constants / weights ----
            qA_s = cpool.tile([128, 128], F32, tag="qA")
            nc.sync.dma_start(out=qA_s[:], in_=qA[:])
            qB_s = cpool.tile([128, 128], F32, tag="qB")
            nc.sync.dma_start(out=qB_s[:], in_=qB[:])
            ut_s = cpool.tile([128, 128], BF16, tag="ut")
            nc.sync.dma_start(out=ut_s[:], in_=ut[:])
            idn_s = cpool.tile([128, 128], F32, tag="idn")
            nc.sync.dma_start(out=idn_s[:], in_=idn[:])
            ones_s = cpool.tile([128, 1], F32, tag="ones")
            nc.sync.dma_start(out=ones_s[:], in_=ones[:])
            ecol_s = cpool.tile([128, 1], F32, tag="ecol")
            nc.sync.dma_start(out=ecol_s[:], in_=ecol[:])
            io64_s = cpool.tile([128, 64], F32, tag="io64")
            nc.sync.dma_start(out=io64_s[:], in_=io64[:])
            gw_s = cpool.tile([128, 6 * 8], F32, tag="gw")
            nc.sync.dma_start(
                out=gw_s[:].rearrange("p (c e) -> p c e", c=6),
                in_=gw[:].rearrange("(c p) e -> p c e", p=128))

            # ---- persistent routing state ----
            cw_all = rtpool.tile([128, NT], F32, tag="cw")
            mask_all = rtpool.tile([128, NT], F32, tag="mask")
            pos_all = rtpool.tile([128, NT], F32, tag="pos")
            acc_pr = rtpool.tile([128, 64], F32, tag="accp")
            nc.vector.memset(acc_pr[:], 0.0)

            # ---- data-parallel gate: my 1024-token slice, then AllGather ----
            lg_loc = rtpool.tile([128, 64], F32, tag="lgloc")
            for t in range(8):
                xg = xgpool.tile([128, 6 * 128], F32, tag="xg", name=f"xg{t}")
                nc.sync.dma_start(
                    out=xg[:].rearrange("p (c t) -> p c t", c=6),
                    in_=xTs[:].rearrange("(c p) n -> p c n", p=128)
                        [:, :, t * 128:(t + 1) * 128])
                lg_ps = psmm.tile([128, 8], F32, tag="mm", name=f"lgps{t}")
                for cj in range(6):
                    nc.tensor.matmul(
                        out=lg_ps[:],
                        lhsT=xg[:, cj * 128:(cj + 1) * 128],
                        rhs=gw_s[:, cj * 8:(cj + 1) * 8],
                        start=(cj == 0), stop=(cj == 5))
                nc.vector.tensor_copy(out=lg_loc[:, t * 8:(t + 1) * 8], in_=lg_ps[:])
            nc.sync.dma_start(out=lgd[:], in_=lg_loc[:])
            nc.gpsimd.collective_compute(
                "AllGather", OP.bypass,
                ins=[lgd[:]], outs=[agg[:]],
                replica_groups=[list(range(NCORE))])

            w1_s = wpool.tile([128, 6 * H], BF16, tag="w1")
            nc.sync.dma_start(
                out=w1_s[:].rearrange("p (c h) -> p c h", c=6),
                in_=w1[:].rearrange("(c p) h -> p c h", p=128))
            w2_s = wpool.tile([128, 24 * C], BF16, tag="w2")
            nc.sync.dma_start(
                out=w2_s[:].rearrange("p (k c) -> p k c", k=24),
                in_=w2[:].rearrange("(k p) c -> p k c", p=128))


            for g in range(NGRP):
                # ============ routing for this group's 8 tiles ============
                lg_grp = smpool.tile([128, 64], F32, tag="lgg", name=f"lgg{g}")
                nc.sync.dma_start(out=lg_grp[:], in_=agg[g * 128:(g + 1) * 128, :])

                lg3 = lg_grp[:].rearrange("p (t e) -> p t e", t=8)
                m1g = smpool.tile([128, 8], F32, tag="m1g", name=f"m1g{g}")
                nc.vector.reduce_max(out=m1g[:], in_=lg3, axis=AX.X)
                m1b = m1g[:][:, :, None].to_broadcast([128, 8, 8])
                eq1 = smpool.tile([128, 64], F32, tag="eq1", name=f"eq1{g}")
                eq13 = eq1[:].rearrange("p (t e) -> p t e", t=8)
                nc.vector.tensor_tensor(out=eq13, in0=lg3, in1=m1b, op=OP.is_equal)
                t64 = smpool.tile([128, 64], F32, tag="t64", name=f"t64{g}")
                nc.vector.tensor_tensor(out=t64[:], in0=eq1[:], in1=io64_s[:], op=OP.mult)
                a1g = smpool.tile([128, 8], F32, tag="a1g", name=f"a1g{g}")
                nc.vector.reduce_max(out=a1g[:], in_=t64[:].rearrange("p (t e) -> p t e", t=8),
                                     axis=AX.X)
                # mask out argmax, find second
                lm = smpool.tile([128, 64], F32, tag="lm", name=f"lm{g}")
                nc.vector.tensor_scalar(out=lm[:], in0=eq1[:], scalar1=-1e30,
                                        scalar2=None, op0=OP.mult)
                nc.vector.tensor_tensor(out=lm[:], in0=lg_grp[:], in1=lm[:], op=OP.add)
                lm3 = lm[:].rearrange("p (t e) -> p t e", t=8)
                m2g = smpool.tile([128, 8], F32, tag="m2g", name=f"m2g{g}")
                nc.vector.reduce_max(out=m2g[:], in_=lm3, axis=AX.X)
                m2b = m2g[:][:, :, None].to_broadcast([128, 8, 8])
                eq2 = smpool.tile([128, 64], F32, tag="eq2", name=f"eq2{g}")
                nc.vector.tensor_tensor(out=eq2[:].rearrange("p (t e) -> p t e", t=8),
                                        in0=lm3, in1=m2b, op=OP.is_equal)
                nc.vector.tensor_tensor(out=t64[:], in0=eq2[:], in1=io64_s[:], op=OP.mult)
                a2g = smpool.tile([128, 8], F32, tag="a2g", name=f"a2g{g}")
                nc.vector.reduce_max(out=a2g[:], in_=t64[:].rearrange("p (t e) -> p t e", t=8),
                                     axis=AX.X)
                # top-2 softmax weights (bulk ACT)
                d21 = smpool.tile([128, 8], F32, tag="d21", name=f"d21{g}")
                nc.vector.tensor_tensor(out=d21[:], in0=m2g[:], in1=m1g[:], op=OP.subtract)
                w2gg = smpool.tile([128, 8], F32, tag="w2gg", name=f"w2gg{g}")
                nc.scalar.activation(out=w2gg[:], in_=d21[:], func=AF.Sigmoid)
                w1gg = smpool.tile([128, 8], F32, tag="w1gg", name=f"w1gg{g}")
                nc.vector.tensor_scalar(out=w1gg[:], in0=w2gg[:], scalar1=-1.0,
                                        scalar2=1.0, op0=OP.mult, op1=OP.add)
                # softmax probs for aux (bulk)
                zs = smpool.tile([128, 64], F32, tag="zs", name=f"zs{g}")
                nc.vector.tensor_tensor(out=zs[:].rearrange("p (t e) -> p t e", t=8),
                                        in0=lg3, in1=m1b, op=OP.subtract)
                ez = smpool.tile([128, 64], F32, tag="ez", name=f"ez{g}")
                nc.scalar.activation(out=ez[:], in_=zs[:], func=AF.Exp)
                den = smpool.tile([128, 8], F32, tag="den", name=f"den{g}")
                nc.vector.reduce_sum(out=den[:], in_=ez[:].rearrange("p (t e) -> p t e", t=8),
                                     axis=AX.X)
                inv = smpool.tile([128, 8], F32, tag="inv", name=f"inv{g}")
                nc.vector.reciprocal(out=inv[:], in_=den[:])
                invb = inv[:][:, :, None].to_broadcast([128, 8, 8])
                pr = smpool.tile([128, 64], F32, tag="pr", name=f"pr{g}")
                nc.vector.tensor_tensor(out=pr[:].rearrange("p (t e) -> p t e", t=8),
                                        in0=ez[:].rearrange("p (t e) -> p t e", t=8),
                                        in1=invb, op=OP.mult)
                nc.vector.tensor_tensor(out=acc_pr[:], in0=acc_pr[:], in1=pr[:], op=OP.add)
                # my-expert mask + combine weight (bulk [128, 8])
                es1 = smpool.tile([128, 8], F32, tag="es1", name=f"es1{g}")
                nc.vector.tensor_scalar(out=es1[:], in0=a1g[:], scalar1=ecol_s[:, :1],
                                        scalar2=None, op0=OP.is_equal)
                es2 = smpool.tile([128, 8], F32, tag="es2", name=f"es2{g}")
                nc.vector.tensor_scalar(out=es2[:], in0=a2g[:], scalar1=ecol_s[:, :1],
                                        scalar2=None, op0=OP.is_equal)
                nc.vector.tensor_tensor(out=mask_all[:, g * 8:(g + 1) * 8],
                                        in0=es1[:], in1=es2[:], op=OP.add)
                nc.vector.tensor_tensor(out=es1[:], in0=es1[:], in1=w1gg[:], op=OP.mult)
                nc.vector.tensor_tensor(out=es2[:], in0=es2[:], in1=w2gg[:], op=OP.mult)
                nc.vector.tensor_tensor(out=cw_all[:, g * 8:(g + 1) * 8],
                                        in0=es1[:], in1=es2[:], op=OP.add)
                # per-group exclusive prefix (tile-local positions)
                mkbf = smpool.tile([128, 8], BF16, tag="mkbf", name=f"mkbf{g}")
                nc.vector.tensor_copy(out=mkbf[:], in_=mask_all[:, g * 8:(g + 1) * 8])
                cum_ps = psmm.tile([128, 8], F32, tag="mm", name=f"cum{g}")
                nc.tensor.matmul(out=cum_ps[:], lhsT=ut_s[:], rhs=mkbf[:],
                                 start=True, stop=True)
                nc.vector.tensor_copy(out=pos_all[:, g * 8:(g + 1) * 8], in_=cum_ps[:])

                # ============ compaction ============
                xc = xcpool.tile([128, 6 * GS], BF16, tag="xc", name=f"xc{g}")
                pts = []
                for s in range(4):          # pairs in group
                    pair = g * 4 + s
                    cp_ps = [psw.tile([128, 3 * PS], F32, tag="w", name=f"cp{g}_{s}_{i}")
                             for i in range(2)]
                    pmws = []
                    for half in range(2):   # tiles in pair
                        T = pair * 2 + half
                        qio = qA_s if half == 0 else qB_s
                        pmw = pmpool.tile([128, PS], F32, tag="pmw", name=f"pmw{T}")
                        nc.vector.tensor_tensor(
                            out=pmw[:],
                            in0=pos_all[:, T:T + 1].to_broadcast([128, PS]),
                            in1=qio[:, :PS], op=OP.is_equal)
                        nc.vector.tensor_scalar(
                            out=pmw[:], in0=pmw[:], scalar1=mask_all[:, T:T + 1],
                            scalar2=None, op0=OP.mult)
                        pmb = pmpool.tile([128, PS], BF16, tag="pmb", name=f"pmb{T}")
                        nc.vector.tensor_copy(out=pmb[:], in_=pmw[:])
                        pmws.append(pmw)

                        xbt = xbpool.tile([128, 768], BF16, tag="xbt", name=f"xbt{T}")
                        nc.sync.dma_start(out=xbt[:], in_=xb[T * 128:(T + 1) * 128, :])
                        for cj in range(6):
                            # one accumulation group per PSUM bank: start on the
                            # first matmul into the bank, stop on the last.
                            nc.tensor.matmul(
                                out=cp_ps[cj // 3][:, (cj % 3) * PS:(cj % 3 + 1) * PS],
                                lhsT=xbt[:, cj * 128:(cj + 1) * 128],
                                rhs=pmb[:],
                                start=(half == 0 and cj % 3 == 0),
                                stop=(half == 1 and cj % 3 == 2))

                    for half in range(2):
                        T = pair * 2 + half
                        psc = pmpool.tile([128, PS], F32, tag="psc", name=f"psc{T}")
                        nc.vector.tensor_scalar(
                            out=psc[:], in0=pmws[half][:], scalar1=cw_all[:, T:T + 1],
                            scalar2=None, op0=OP.mult)
                        pt_ps = psmm.tile([PS, 128], F32, tag="mm", name=f"ptps{T}")
                        nc.tensor.transpose(out=pt_ps[:], in_=psc[:], identity=idn_s[:])
                        ptb = ptpool.tile([PS, 128], BF16, tag="ptb", name=f"ptb{T}")
                        nc.vector.tensor_copy(out=ptb[:], in_=pt_ps[:])
                        pts.append(ptb)

                    for w in range(2):
                        nc.vector.tensor_copy(
                            out=xc[:].rearrange("p (c q) -> p c q", c=6)
                                [:, w * 3:(w + 1) * 3, s * PS:(s + 1) * PS],
                            in_=cp_ps[w][:].rearrange("p (c q) -> p c q", c=3))

                # ============ GEMM1 + gelu ============
                ht = htpool.tile([128, 24 * GS], BF16, tag="ht", name=f"ht{g}")
                for hk in range(24):
                    g1 = psmm.tile([128, GS], F32, tag="mm", name=f"g1_{g}_{hk}")
                    for cj in range(6):
                        nc.tensor.matmul(
                            out=g1[:],
                            lhsT=w1_s[:, cj * H + hk * 128: cj * H + (hk + 1) * 128],
                            rhs=xc[:, cj * GS:(cj + 1) * GS],
                            start=(cj == 0), stop=(cj == 5))
                    nc.scalar.activation(out=ht[:, hk * GS:(hk + 1) * GS], in_=g1[:],
                                         func=AF.Gelu)

                # ============ GEMM2 + combine ============
                for s in range(4):
                    pair = g * 4 + s
                    y_ps = [psw.tile([PS, 384], F32, tag="w", name=f"y{g}_{s}_{i}")
                            for i in range(2)]
                    for hk in range(24):
                        for w in range(2):
                            nc.tensor.matmul(
                                out=y_ps[w][:, :],
                                lhsT=ht[:, hk * GS + s * PS: hk * GS + (s + 1) * PS],
                                rhs=w2_s[:, hk * C + w * 384: hk * C + (w + 1) * 384],
                                start=(hk == 0), stop=(hk == 23))
                    y_sb = ypool.tile([PS, 768], BF16, tag="ysb", name=f"ysb{pair}")
                    nc.vector.tensor_copy(out=y_sb[:, 0:384], in_=y_ps[0][:])
                    nc.vector.tensor_copy(out=y_sb[:, 384:768], in_=y_ps[1][:])

                    for half in range(2):
                        T = pair * 2 + half
                        ptb = pts[s * 2 + half]
                        oc_ps = [psw.tile([128, 384], F32, tag="w", name=f"oc{T}_{i}")
                                 for i in range(2)]
                        for w in range(2):
                            nc.tensor.matmul(
                                out=oc_ps[w][:],
                                lhsT=ptb[:],
                                rhs=y_sb[:, w * 384:(w + 1) * 384],
                                start=True, stop=True)
                        o_sb = opool.tile([128, 768], BF16, tag="osb", name=f"osb{T}")
                        nc.vector.tensor_copy(out=o_sb[:, 0:384], in_=oc_ps[0][:])
                        nc.vector.tensor_copy(out=o_sb[:, 384:768], in_=oc_ps[1][:])
                        ch = next(i for i, gs in enumerate(RS_GROUPS) if T // 8 in gs)
                        Tc = T - RS_GROUPS[ch][0] * 8
                        nc.sync.dma_start(
                            out=parts[ch][Tc * 128:(Tc + 1) * 128, :],
                            in_=o_sb[:])

                # ============ chunked cross-core combine ============
                for ci, gs in enumerate(RS_GROUPS):
                    if g == gs[-1]:
                        off = RS_GROUPS[ci][0] * 128
                        sz = 128 * len(gs)
                        nc.gpsimd.collective_compute(
                            "ReduceScatter", OP.add,
                            ins=[parts[ci][:]],
                            outs=[rs_outs[ci][:]],
                            replica_groups=[list(range(NCORE))])
                        nc.sync.dma_start(out=out_sh[off:off + sz, :],
                                          in_=rs_outs[ci][:])

            # ---- aux loss finalize ----
            ax_ps = psmm.tile([1, 64], F32, tag="mm")
            nc.tensor.matmul(out=ax_ps[:], lhsT=ones_s[:], rhs=acc_pr[:],
                             start=True, stop=True)
            frv = smpool.tile([1, 64], F32, tag="frv")
            nc.vector.tensor_scalar(out=frv[:], in0=ax_ps[:], scalar1=1.0 / N,
                                    scalar2=None, op0=OP.mult)
            fre = smpool.tile([1, 8], F32, tag="fre")
            nc.vector.reduce_sum(out=fre[:], in_=frv[:].rearrange("p (t e) -> p e t", t=8),
                                 axis=AX.X)
            nc.vector.tensor_tensor(out=fre[:], in0=fre[:], in1=fre[:], op=OP.mult)
            axv = smpool.tile([1, 1], F32, tag="axv")
            nc.vector.reduce_sum(out=axv[:], in_=fre[:], axis=AX.X)
            nc.vector.tensor_scalar(out=axv[:], in0=axv[:], scalar1=0.01 * E,
                                    scalar2=None, op0=OP.mult)
            nc.sync.dma_start(out=aux_o[:], in_=axv[:])



    nc.compile()
    return nc


def make_consts():
    qA = np.tile(np.arange(128, dtype=np.float32), (128, 1))
    qB = qA - float(CAP)
    ut = np.triu(np.ones((128, 128), dtype=np.float32), k=1)
    idn = np.eye(128, dtype=np.float32)
    ones = np.ones((128, 1), dtype=np.float32)
    io64 = np.tile(np.tile(np.arange(8, dtype=np.float32), 8), (128, 1))
    return qA, qB, ut, idn, ones, io64


def kernel(x, gate_w, w1, w2):
    x = np.asarray(x, dtype=np.float32)
    gate_w = np.asarray(gate_w, dtype=np.float32)
    w1 = np.asarray(w1, dtype=np.float32)
    w2 = np.asarray(w2, dtype=np.float32)
    B, T_, C_ = x.shape
    xt = np.ascontiguousarray(x.reshape(-1, C_))

    # capacity sanity check (cheap host-side gate replication)
    lgh = xt @ gate_w
    top2 = np.argpartition(-lgh, 2, axis=1)[:, :2]
    cnt = np.zeros((NT, E), np.int64)
    for k in range(2):
        np.add.at(cnt, (np.arange(N) // 128, top2[:, k]), 1)
    assert cnt.max() <= CAP, f"tile capacity overflow: {cnt.max()} > {CAP}"

    xT = np.ascontiguousarray(xt.T)
    xTslices = [np.ascontiguousarray(xT[:, 1024 * e:1024 * (e + 1)]) for e in range(NCORE)]
    xbf = xt.astype(ml_dtypes.bfloat16)
    w1b = w1.astype(ml_dtypes.bfloat16)
    w2b = w2.astype(ml_dtypes.bfloat16)

    qA, qB, ut, idn, ones, io64 = make_consts()
    utb = ut.astype(ml_dtypes.bfloat16)

    nc = build_program()

    in_maps = []
    for e in range(NCORE):
        in_maps.append({
            "xTs": xTslices[e], "xb": xbf,
            "w1": np.ascontiguousarray(w1b[e]),
            "w2": np.ascontiguousarray(w2b[e]),
            "gw": gate_w,
            "qA": qA, "qB": qB, "ut": utb, "idn": idn, "ones": ones,
            "ecol": np.full((128, 1), float(e), dtype=np.float32),
            "io64": io64,
        })

    res = run_bass_kernel_spmd(nc, in_maps, core_ids=list(range(NCORE)),
                               **_trace_kwargs())
    _stash_results(res)

    # reassemble: chunked RS gives core i, chunk c -> global rows 2048c+256i..+256
    out = np.empty((N, C), np.float32)
    for i in range(NCORE):
        sh = res.results[i]["out_sh"]
        for gs in RS_GROUPS:
            base = gs[0] * 1024              # global row of chunk start
            csz = 1024 * len(gs) // NCORE    # rows per core in this chunk
            shoff = gs[0] * 128              # rows into rs_out/out_sh
            out[base + csz * i: base + csz * (i + 1)] = \
                np.asarray(sh[shoff: shoff + csz], dtype=np.float32)
    out = out.reshape(B, T_, C_)
    aux = np.float32(res.results[0]["aux_o"][0, 0])
    return out, aux


_TRACE = {"enabled": False}
_LAST = {}


def _trace_kwargs():
    if not _TRACE["enabled"]:
        return {}
    import concourse.bass_utils as bu
    bu.upload_artifacts = lambda d: d
    m = types.ModuleType("antenv.axon_hooks")
    m._hook = None
    def set_hook(h): m._hook = h
    def get_hook(): return m._hook
    m.set_axon_ntff_profile_hook = set_hook
    m.get_axon_ntff_profile_hook = get_hook
    sys.modules['antenv.axon_hooks'] = m
    import antenv
    antenv.axon_hooks = m
    from trn_agent_boot.trn_boot import _ntff_profile_via_ctypes
    set_hook(_ntff_profile_via_ctypes('/opt/axon/libaxon_pjrt.so'))
    return {"trace": True}


def _stash_results(res):
    _LAST["exec_time_ns"] = res.exec_time_ns
    _LAST["mean_exec_time_ns"] = res.mean_exec_time_ns
    if res.instructions_and_trace:
        _LAST["trace_path"] = res.instructions_and_trace[1]


# revision 27
# speedup vs baseline: 1.0684x; 1.0221x over previous
import sys, os, types

sys.path.insert(0, '/opt/trn_rl_repo')

import numpy as np
import ml_dtypes
from concourse import bass, bacc, mybir
import concourse.tile as tile
from concourse.bass_utils import run_bass_kernel_spmd

dt = mybir.dt
F32 = dt.float32
BF16 = dt.bfloat16
AX = mybir.AxisListType
OP = mybir.AluOpType
AF = mybir.ActivationFunctionType

E = 8            # experts == cores
N = 8192         # tokens
C = 768          # model dim
H = 3072         # hidden dim
NCORE = 8
NT = N // 128    # 64 token tiles
NGRP = NT // 8   # 8 groups of 512 slots
CAP = 48         # compacted slots per 128-token tile
PS = 2 * CAP     # slots per pair
GS = 8 * CAP     # slots per group
RS_GROUPS = [[0, 1], [2, 3], [4, 5], [6], [7]]  # groups per RS chunk
NRS = len(RS_GROUPS)


def build_program():
    nc = bacc.Bacc("TRN2", target_bir_lowering=False, debug=False,
                   num_devices=NCORE)

    xTs = nc.dram_tensor("xTs", [C, N // NCORE], F32, kind="ExternalInput")
    xb = nc.dram_tensor("xb", [N, C], BF16, kind="ExternalInput")
    w1 = nc.dram_tensor("w1", [C, H], BF16, kind="ExternalInput")
    w2 = nc.dram_tensor("w2", [H, C], BF16, kind="ExternalInput")
    gw = nc.dram_tensor("gw", [C, E], F32, kind="ExternalInput")
    # constants
    qA = nc.dram_tensor("qA", [128, 128], F32, kind="ExternalInput")   # row iota 0..127
    qB = nc.dram_tensor("qB", [128, 128], F32, kind="ExternalInput")   # row iota -64..63
    ut = nc.dram_tensor("ut", [128, 128], BF16, kind="ExternalInput")  # U[q,p]=1 if q<p
    idn = nc.dram_tensor("idn", [128, 128], F32, kind="ExternalInput")  # identity
    ones = nc.dram_tensor("ones", [128, 1], F32, kind="ExternalInput")
    ecol = nc.dram_tensor("ecol", [128, 1], F32, kind="ExternalInput")  # expert id per core
    io64 = nc.dram_tensor("io64", [128, 64], F32, kind="ExternalInput")  # rows tile(0..7, 8)

    out_sh = nc.dram_tensor("out_sh", [N // 8, C], BF16, kind="ExternalOutput")
    aux_o = nc.dram_tensor("aux_o", [1, 1], F32, kind="ExternalOutput")

    parts = [nc.dram_tensor(f"part{c}", [1024 * len(gs), C], BF16) for c, gs in enumerate(RS_GROUPS)]
    rs_outs = [nc.dram_tensor(f"rs_out{c}", [128 * len(gs), C], BF16)
               for c, gs in enumerate(RS_GROUPS)]
    lgd = nc.dram_tensor("lgd", [128, 64], F32)
    agg = nc.dram_tensor("agg", [128 * NCORE, 64], F32, addr_space="Shared")

    with tile.TileContext(nc) as tc:
        with (
            tc.tile_pool(name="const", bufs=1) as cpool,
            tc.tile_pool(name="wts", bufs=1) as wpool,
            tc.tile_pool(name="rt", bufs=1) as rtpool,        # persistent routing
            tc.tile_pool(name="xg", bufs=4) as xgpool,
            tc.tile_pool(name="xbt", bufs=8) as xbpool,
            tc.tile_pool(name="sm", bufs=4) as smpool,        # small per-group work
            tc.tile_pool(name="pm", bufs=4) as pmpool,        # P matrices
            tc.tile_pool(name="pt", bufs=10) as ptpool,       # scaled P^T (live per group)
            tc.tile_pool(name="xc", bufs=2) as xcpool,        # per-group compact x
            tc.tile_pool(name="ht", bufs=3) as htpool,        # per-group hidden
            tc.tile_pool(name="ysb", bufs=3) as ypool,
            tc.tile_pool(name="osb", bufs=3) as opool,
            tc.tile_pool(name="ps_mm", bufs=3, space="PSUM") as psmm,    # 1-bank units
            tc.tile_pool(name="ps_w", bufs=5, space="PSUM") as psw,      # [*,384] units
        ):
            # ---- load constants / weights ----
            qA_s = cpool.tile([128, 128], F32, tag="qA")
            nc.sync.dma_start(out=qA_s[:], in_=qA[:])
            qB_s = cpool.tile([128, 128], F32, tag="qB")
            nc.sync.dma_start(out=qB_s[:], in_=qB[:])
            ut_s = cpool.tile([128, 128], BF16, tag="ut")
            nc.sync.dma_start(out=ut_s[:], in_=ut[:])
            idn_s = cpool.tile([128, 128], F32, tag="idn")
            nc.sync.dma_start(out=idn_s[:], in_=idn[:])
            ones_s = cpool.tile([128, 1], F32, tag="ones")
            nc.sync.dma_start(out=ones_s[:], in_=ones[:])
            ecol_s = cpool.tile([128, 1], F32, tag="ecol")
            nc.sync.dma_start(out=ecol_s[:], in_=ecol[:])
            io64_s = cpool.tile([128, 64], F32, tag="io64")
            nc.sync.dma_start(out=io64_s[:], in_=io64[:])
            gw_s = cpool.tile([128, 6 * 8], F32, tag="gw")
            nc.sync.dma_start(
                out=gw_s[:].rearrange("p (c e) -> p c e", c=6),
                in_=gw[:].rearrange("(c p) e -> p c e", p=128))

            # ---- persistent routing state ----
            cw_all = rtpool.tile([128, NT], F32, tag="cw")
            mask_all = rtpool.tile([128, NT], F32, tag="mask")
            pos_all = rtpool.tile([128, NT], F32, tag="pos")
            acc_pr = rtpool.tile([128, 64], F32, tag="accp")
            nc.vector.memset(acc_pr[:], 0.0)

            # ---- data-parallel gate: my 1024-token slice, then AllGather ----
            lg_loc = rtpool.tile([128, 64], F32, tag="lgloc")
            for t in range(8):
                xg = xgpool.tile([128, 6 * 128], F32, tag="xg", name=f"xg{t}")
                nc.sync.dma_start(
                    out=xg[:].rearrange("p (c t) -> p c t", c=6),
                    in_=xTs[:].rearrange("(c p) n -> p c n", p=128)
                        [:, :, t * 128:(t + 1) * 128])
                lg_ps = psmm.tile([128, 8], F32, tag="mm", name=f"lgps{t}")
                for cj in range(6):
                    nc.tensor.matmul(
                        out=lg_ps[:],
                        lhsT=xg[:, cj * 128:(cj + 1) * 128],
                        rhs=gw_s[:, cj * 8:(cj + 1) * 8],
                        start=(cj == 0), stop=(cj == 5))
                nc.vector.tensor_copy(out=lg_loc[:, t * 8:(t + 1) * 8], in_=lg_ps[:])
            nc.sync.dma_start(out=lgd[:], in_=lg_loc[:])
            nc.gpsimd.collective_compute(
                "AllGather", OP.bypass,
                ins=[lgd[:]], outs=[agg[:]],
                replica_groups=[list(range(NCORE))])

            w1_s = wpool.tile([128, 6 * H], BF16, tag="w1")
            nc.sync.dma_start(
                out=w1_s[:].rearrange("p (c h) -> p c h", c=6),
                in_=w1[:].rearrange("(c p) h -> p c h", p=128))
            w2_s = wpool.tile([128, 24 * C], BF16, tag="w2")
            nc.sync.dma_start(
                out=w2_s[:].rearrange("p (k c) -> p k c", k=24),
                in_=w2[:].rearrange("(k p) c -> p k c", p=128))


            for g in range(NGRP):
                # ============ routing for this group's 8 tiles ============
                lg_grp = smpool.tile([128, 64], F32, tag="lgg", name=f"lgg{g}")
                nc.sync.dma_start(out=lg_grp[:], in_=agg[g * 128:(g + 1) * 128, :])

                lg3 = lg_grp[:].rearrange("p (t e) -> p t e", t=8)
                m1g = smpool.tile([128, 8], F32, tag="m1g", name=f"m1g{g}")
                nc.vector.reduce_max(out=m1g[:], in_=lg3, axis=AX.X)
                m1b = m1g[:][:, :, None].to_broadcast([128, 8, 8])
                eq1 = smpool.tile([128, 64], F32, tag="eq1", name=f"eq1{g}")
                eq13 = eq1[:].rearrange("p (t e) -> p t e", t=8)
                nc.vector.tensor_tensor(out=eq13, in0=lg3, in1=m1b, op=OP.is_equal)
                t64 = smpool.tile([128, 64], F32, tag="t64", name=f"t64{g}")
                nc.vector.tensor_tensor(out=t64[:], in0=eq1[:], in1=io64_s[:], op=OP.mult)
                a1g = smpool.tile([128, 8], F32, tag="a1g", name=f"a1g{g}")
                nc.vector.reduce_max(out=a1g[:], in_=t64[:].rearrange("p (t e) -> p t e", t=8),
                                     axis=AX.X)
                # mask out argmax, find second
                lm = smpool.tile([128, 64], F32, tag="lm", name=f"lm{g}")
                nc.vector.tensor_scalar(out=lm[:], in0=eq1[:], scalar1=-1e30,
                                        scalar2=None, op0=OP.mult)
                nc.vector.tensor_tensor(out=lm[:], in0=lg_grp[:], in1=lm[:], op=OP.add)
                lm3 = lm[:].rearrange("p (t e) -> p t e", t=8)
                m2g = smpool.tile([128, 8], F32, tag="m2g", name=f"m2g{g}")
                nc.vector.reduce_max(out=m2g[:], in_=lm3, axis=AX.X)
                m2b = m2g[:][:, :, None].to_broadcast([128, 8, 8])
                eq2 = smpool.tile([128, 64], F32, tag="eq2", name=f"eq2{g}")
                nc.vector.tensor_tensor(out=eq2[:].rearrange("p (t e) -> p t e", t=8),
                                        in0=lm3, in1=m2b, op=OP.is_equal)
                nc.vector.tensor_tensor(out=t64[:], in0=eq2[:], in1=io64_s[:], op=OP.mult)
                a2g = smpool.tile([128, 8], F32, tag="a2g", name=f"a2g{g}")
                nc.vector.reduce_max(out=a2g[:], in_=t64[:].rearrange("p (t e) -> p t e", t=8),
                                     axis=AX.X)
                # top-2 softmax weights (bulk ACT)
                d21 = smpool.tile([128, 8], F32, tag="d21", name=f"d21{g}")
                nc.vector.tensor_tensor(out=d21[:], in0=m2g[:], in1=m1g[:], op=OP.subtract)
                w2gg = smpool.tile([128, 8], F32, tag="w2gg", name=f"w2gg{g}")
                nc.scalar.activation(out=w2gg[:], in_=d21[:], func=AF.Sigmoid)
                w1gg = smpool.tile([128, 8], F32, tag="w1gg", name=f"w1gg{g}")
                nc.vector.tensor_scalar(out=w1gg[:], in0=w2gg[:], scalar1=-1.0,
                                        scalar2=1.0, op0=OP.mult, op1=OP.add)
                # softmax probs for aux (bulk)
                zs = smpool.tile([128, 64], F32, tag="zs", name=f"zs{g}")
                nc.vector.tensor_tensor(out=zs[:].rearrange("p (t e) -> p t e", t=8),
                                        in0=lg3, in1=m1b, op=OP.subtract)
                ez = smpool.tile([128, 64], F32, tag="ez", name=f"ez{g}")
                nc.scalar.activation(out=ez[:], in_=zs[:], func=AF.Exp)
                den = smpool.tile([128, 8], F32, tag="den", name=f"den{g}")
                nc.vector.reduce_sum(out=den[:], in_=ez[:].rearrange("p (t e) -> p t e", t=8),
                                     axis=AX.X)
                inv = smpool.tile([128, 8], F32, tag="inv", name=f"inv{g}")
                nc.vector.reciprocal(out=inv[:], in_=den[:])
                invb = inv[:][:, :, None].to_broadcast([128, 8, 8])
                pr = smpool.tile([128, 64], F32, tag="pr", name=f"pr{g}")
                nc.vector.tensor_tensor(out=pr[:].rearrange("p (t e) -> p t e", t=8),
                                        in0=ez[:].rearrange("p (t e) -> p t e", t=8),
                                        in1=invb, op=OP.mult)
                nc.vector.tensor_tensor(out=acc_pr[:], in0=acc_pr[:], in1=pr[:], op=OP.add)
                # my-expert mask + combine weight (bulk [128, 8])
                es1 = smpool.tile([128, 8], F32, tag="es1", name=f"es1{g}")
                nc.vector.tensor_scalar(out=es1[:], in0=a1g[:], scalar1=ecol_s[:, :1],
                                        scalar2=None, op0=OP.is_equal)
                es2 = smpool.tile([128, 8], F32, tag="es2", name=f"es2{g}")
                nc.vector.tensor_scalar(out=es2[:], in0=a2g[:], scalar1=ecol_s[:, :1],
                                        scalar2=None, op0=OP.is_equal)
                nc.vector.tensor_tensor(out=mask_all[:, g * 8:(g + 1) * 8],
                                        in0=es1[:], in1=es2[:], op=OP.add)
                nc.vector.tensor_tensor(out=es1[:], in0=es1[:], in1=w1gg[:], op=OP.mult)
                nc.vector.tensor_tensor(out=es2[:], in0=es2[:], in1=w2gg[:], op=OP.mult)
                nc.vector.tensor_tensor(out=cw_all[:, g * 8:(g + 1) * 8],
                                        in0=es1[:], in1=es2[:], op=OP.add)
                # per-group exclusive prefix (tile-local positions)
                mkbf = smpool.tile([128, 8], BF16, tag="mkbf", name=f"mkbf{g}")
                nc.vector.tensor_copy(out=mkbf[:], in_=mask_all[:, g * 8:(g + 1) * 8])
                cum_ps = psmm.tile([128, 8], F32, tag="mm", name=f"cum{g}")
                nc.tensor.matmul(out=cum_ps[:], lhsT=ut_s[:], rhs=mkbf[:],
                                 start=True, stop=True)
                nc.vector.tensor_copy(out=pos_all[:, g * 8:(g + 1) * 8], in_=cum_ps[:])

                # ============ compaction ============
                xc = xcpool.tile([128, 6 * GS], BF16, tag="xc", name=f"xc{g}")
                pts = []
                for s in range(4):          # pairs in group
                    pair = g * 4 + s
                    cp_ps = [psw.tile([128, 3 * PS], F32, tag="w", name=f"cp{g}_{s}_{i}")
                             for i in range(2)]
                    pmws = []
                    for half in range(2):   # tiles in pair
                        T = pair * 2 + half
                        qio = qA_s if half == 0 else qB_s
                        pmw = pmpool.tile([128, PS], F32, tag="pmw", name=f"pmw{T}")
                        nc.vector.tensor_tensor(
                            out=pmw[:],
                            in0=pos_all[:, T:T + 1].to_broadcast([128, PS]),
                            in1=qio[:, :PS], op=OP.is_equal)
                        nc.vector.tensor_scalar(
                            out=pmw[:], in0=pmw[:], scalar1=mask_all[:, T:T + 1],
                            scalar2=None, op0=OP.mult)
                        pmb = pmpool.tile([128, PS], BF16, tag="pmb", name=f"pmb{T}")
                        nc.vector.tensor_copy(out=pmb[:], in_=pmw[:])
                        pmws.append(pmw)

                        xbt = xbpool.tile([128, 768], BF16, tag="xbt", name=f"xbt{T}")
                        nc.sync.dma_start(out=xbt[:], in_=xb[T * 128:(T + 1) * 128, :])
                        for cj in range(6):
                            # one accumulation group per PSUM bank: start on the
                            # first matmul into the bank, stop on the last.
                            nc.tensor.matmul(
                                out=cp_ps[cj // 3][:, (cj % 3) * PS:(cj % 3 + 1) * PS],
                                lhsT=xbt[:, cj * 128:(cj + 1) * 128],
                                rhs=pmb[:],
                                start=(half == 0 and cj % 3 == 0),
                                stop=(half == 1 and cj % 3 == 2))

                    for half in range(2):
                        T = pair * 2 + half
                        psc = pmpool.tile([128, PS], F32, tag="psc", name=f"psc{T}")
                        nc.vector.tensor_scalar(
                            out=psc[:], in0=pmws[half][:], scalar1=cw_all[:, T:T + 1],
                            scalar2=None, op0=OP.mult)
                        pt_ps = psmm.tile([PS, 128], F32, tag="mm", name=f"ptps{T}")
                        nc.tensor.transpose(out=pt_ps[:], in_=psc[:], identity=idn_s[:])
                        ptb = ptpool.tile([PS, 128], BF16, tag="ptb", name=f"ptb{T}")
                        nc.vector.tensor_copy(out=ptb[:], in_=pt_ps[:])
                        pts.append(ptb)

                    for w in range(2):
                        nc.vector.tensor_copy(
                            out=xc[:].rearrange("p (c q) -> p c q", c=6)
                                [:, w * 3:(w + 1) * 3, s * PS:(s + 1) * PS],
                            in_=cp_ps[w][:].rearrange("p (c q) -> p c q", c=3))

                # ============ GEMM1 + gelu ============
                ht = htpool.tile([128, 24 * GS], BF16, tag="ht", name=f"ht{g}")
                for hk in range(24):
                    g1 = psmm.tile([128, GS], F32, tag="mm", name=f"g1_{g}_{hk}")
                    for cj in range(6):
                        nc.tensor.matmul(
                            out=g1[:],
                            lhsT=w1_s[:, cj * H + hk * 128: cj * H + (hk + 1) * 128],
                            rhs=xc[:, cj * GS:(cj + 1) * GS],
                            start=(cj == 0), stop=(cj == 5))
                    nc.scalar.activation(out=ht[:, hk * GS:(hk + 1) * GS], in_=g1[:],
                                         func=AF.Gelu)

                # ============ GEMM2 + combine ============
                for s in range(4):
                    pair = g * 4 + s
                    y_ps = [psw.tile([PS, 384], F32, tag="w", name=f"y{g}_{s}_{i}")
                            for i in range(2)]
                    for hk in range(24):
                        for w in range(2):
                            nc.tensor.matmul(
                                out=y_ps[w][:, :],
                                lhsT=ht[:, hk * GS + s * PS: hk * GS + (s + 1) * PS],
                                rhs=w2_s[:, hk * C + w * 384: hk * C + (w + 1) * 384],
                                start=(hk == 0), stop=(hk == 23))
                    y_sb = ypool.tile([PS, 768], BF16, tag="ysb", name=f"ysb{pair}")
                    nc.vector.tensor_copy(out=y_sb[:, 0:384], in_=y_ps[0][:])
                    nc.vector.tensor_copy(out=y_sb[:, 384:768], in_=y_ps[1][:])

                    for half in range(2):
                        T = pair * 2 + half
                        ptb = pts[s * 2 + half]
                        oc_ps = [psw.tile([128, 384], F32, tag="w", name=f"oc{T}_{i}")
                                 for i in range(2)]
                        for w in range(2):
                            nc.tensor.matmul(
                                out=oc_ps[w][:],
                                lhsT=ptb[:],
                                rhs=y_sb[:, w * 384:(w + 1) * 384],
                                start=True, stop=True)
                        o_sb = opool.tile([128, 768], BF16, tag="osb", name=f"osb{T}")
                        nc.vector.tensor_copy(out=o_sb[:, 0:384], in_=oc_ps[0][:])
                        nc.vector.tensor_copy(out=o_sb[:, 384:768], in_=oc_ps[1][:])
                        ch = next(i for i, gs in enumerate(RS_GROUPS) if T // 8 in gs)
                        Tc = T - RS_GROUPS[ch][0] * 8
                        nc.sync.dma_start(
                            out=parts[ch][Tc * 128:(Tc + 1) * 128, :],
                            in_=o_sb[:])

                # ============ chunked cross-core combine ============
                for ci, gs in enumerate(RS_GROUPS):
                    if g == gs[-1]:
                        off = RS_GROUPS[ci][0] * 128
                        sz = 128 * len(gs)
                        nc.gpsimd.collective_compute(
                            "ReduceScatter", OP.add,
                            ins=[parts[ci][:]],
                            outs=[rs_outs[ci][:]],
                            replica_groups=[list(range(NCORE))])
                        nc.sync.dma_start(out=out_sh[off:off + sz, :],
                                          in_=rs_outs[ci][:])

            # ---- aux loss finalize ----
            ax_ps = psmm.tile([1, 64], F32, tag="mm")
            nc.tensor.matmul(out=ax_ps[:], lhsT=ones_s[:], rhs=acc_pr[:],
                             start=True, stop=True)
            frv = smpool.tile([1, 64], F32, tag="frv")
            nc.vector.tensor_scalar(out=frv[:], in0=ax_ps[:], scalar1=1.0 / N,
                                    scalar2=None, op0=OP.mult)
            fre = smpool.tile([1, 8], F32, tag="fre")
            nc.vector.reduce_sum(out=fre[:], in_=frv[:].rearrange("p (t e) -> p e t", t=8),
                                 axis=AX.X)
            nc.vector.tensor_tensor(out=fre[:], in0=fre[:], in1=fre[:], op=OP.mult)
            axv = smpool.tile([1, 1], F32, tag="axv")
            nc.vector.reduce_sum(out=axv[:], in_=fre[:], axis=AX.X)
            nc.vector.tensor_scalar(out=axv[:], in0=axv[:], scalar1=0.01 * E,
                                    scalar2=None, op0=OP.mult)
            nc.sync.dma_start(out=aux_o[:], in_=axv[:])



    nc.compile()
    return nc


def make_consts():
    qA = np.tile(np.arange(128, dtype=np.float32), (128, 1))
    qB = qA - float(CAP)
    ut = np.triu(np.ones((128, 128), dtype=np.float32), k=1)
    idn = np.eye(128, dtype=np.float32)
    ones = np.ones((128, 1), dtype=np.float32)
    io64 = np.tile(np.tile(np.arange(8, dtype=np.float32), 8), (128, 1))
    return qA, qB, ut, idn, ones, io64


def kernel(x, gate_w, w1, w2):
    x = np.asarray(x, dtype=np.float32)
    gate_w = np.asarray(gate_w, dtype=np.float32)
    w1 = np.asarray(w1, dtype=np.float32)
    w2 = np.asarray(w2, dtype=np.float32)
    B, T_, C_ = x.shape
    xt = np.ascontiguousarray(x.reshape(-1, C_))

    # capacity sanity check (cheap host-side gate replication)
    lgh = xt @ gate_w
    top2 = np.argpartition(-lgh, 2, axis=1)[:, :2]
    cnt = np.zeros((NT, E), np.int64)
    for k in range(2):
        np.add.at(cnt, (np.arange(N) // 128, top2[:, k]), 1)
    assert cnt.max() <= CAP, f"tile capacity overflow: {cnt.max()} > {CAP}"

    xT = np.ascontiguousarray(xt.T)
    xTslices = [np.ascontiguousarray(xT[:, 1024 * e:1024 * (e + 1)]) for e in range(NCORE)]
    xbf = xt.astype(ml_dtypes.bfloat16)
    w1b = w1.astype(ml_dtypes.bfloat16)
    w2b = w2.astype(ml_dtypes.bfloat16)

    qA, qB, ut, idn, ones, io64 = make_consts()
    utb = ut.astype(ml_dtypes.bfloat16)

    nc = build_program()

    in_maps = []
    for e in range(NCORE):
        in_maps.append({
            "xTs": xTslices[e], "xb": xbf,
            "w1": np.ascontiguousarray(w1b[e]),
            "w2": np.ascontiguousarray(w2b[e]),
            "gw": gate_w,
            "qA": qA, "qB": qB, "ut": utb, "idn": idn, "ones": ones,
            "ecol": np.full((128, 1), float(e), dtype=np.float32),
            "io64": io64,
        })

    res = run_bass_kernel_spmd(nc, in_maps, core_ids=list(range(NCORE)),
                               **_trace_kwargs())
    _stash_results(res)

    # reassemble: chunked RS gives core i, chunk c -> global rows 2048c+256i..+256
    out = np.empty((N, C), np.float32)
    for i in range(NCORE):
        sh = res.results[i]["out_sh"]
        for gs in RS_GROUPS:
            base = gs[0] * 1024              # global row of chunk start
            csz = 1024 * len(gs) // NCORE    # rows per core in this chunk
            shoff = gs[0] * 128              # rows into rs_out/out_sh
            out[base + csz * i: base + csz * (i + 1)] = \
                np.asarray(sh[shoff: shoff + csz], dtype=np.float32)
    out = out.reshape(B, T_, C_)
    aux = np.float32(res.results[0]["aux_o"][0, 0])
    return out, aux


_TRACE = {"enabled": False}
_LAST = {}


def _trace_kwargs():
    if not _TRACE["enabled"]:
        return {}
    import concourse.bass_utils as bu
    bu.upload_artifacts = lambda d: d
    m = types.ModuleType("antenv.axon_hooks")
    m._hook = None
    def set_hook(h): m._hook = h
    def get_hook(): return m._hook
    m.set_axon_ntff_profile_hook = set_hook
    m.get_axon_ntff_profile_hook = get_hook
    sys.modules['antenv.axon_hooks'] = m
    import antenv
    antenv.axon_hooks = m
    from trn_agent_boot.trn_boot import _ntff_profile_via_ctypes
    set_hook(_ntff_profile_via_ctypes('/opt/axon/libaxon_pjrt.so'))
    return {"trace": True}


def _stash_results(res):
    _LAST["exec_time_ns"] = res.exec_time_ns
    _LAST["mean_exec_time_ns"] = res.mean_exec_time_ns
    if res.instructions_and_trace:
        _LAST["trace_path"] = res.instructions_and_trace[1]
